# revision 58
# baseline (speedup 1.0000x reference)
"""DynamicGraphConv Trainium2 kernel (8 NeuronCores, SPMD).

Edges are sorted by destination and sharded across cores at node-aligned
equal-edge boundaries, so each core owns a disjoint slice of the output
and no cross-core collective is needed.

Host prep (index bookkeeping + first MLP layer):
  - h = relu(edge_feat @ W1 + b1) computed host-side, shipped as
    hp^T [65, ep] bf16 (row 64 = ones, folding b2 via W2p's extra row).
  - x[src] gathered per-edge, shipped edge-major ([128, st, 16] bf16).
  - Edge runs (one per destination node) are bin-packed (FFD over degree
    buckets) into 128-edge subtiles, <=32 node-slots each; the one-hot
    seg matrices [128, st, 32] bf16 are built host-side.
  - W2 columns permuted to (o, i) order; b2 appended as row 64.

Per-core device pipeline (v2, edge-major, 512-edge tiles = 4 subtiles):
  mm2:   wt[128e, 256] = hp[:,q].T @ W2p          (PE x4, PSUM fp32)
  drain: wts = bf16(wt)                           (ACT cols<240, DVE rest)
  prod:  pr[e,(o,i)] = wts * xs[e,i]              (DVE 2x q0-2, GPSIMD q3;
         xs broadcast along o via stride-0 AP — no replication DMA)
  segmm: z[32q:32q+32, 256] = seg_q.T @ pr_q      (PE x4, one PSUM bank
         per 2 tiles, tile_position column offsets)
  red:   sl2[:, tile, :] = sum_i z                (DVE tensor_reduce X,
         node-level: 10x fewer rows than edges)
  out:   sl2 [128, 8, 16] streamed to DRAM per group.

Host finalize: out[n] = sl2[slot(n)] * A[n] + B[n] (pure permutation;
A = 1/max(cnt,1) masked, B = bias or x[n]+bias).
"""

import os
import sys

import numpy as np

for _p in ("/opt/trn_rl_repo",):
    if _p not in sys.path and os.path.isdir(_p):
        sys.path.insert(0, _p)

import ml_dtypes  # noqa: E402

import concourse.bass as bass  # noqa: E402
import concourse.tile as tile  # noqa: E402
from concourse import bacc  # noqa: E402
from concourse import library_config  # noqa: E402
from concourse import mybir  # noqa: E402
from concourse._compat import with_exitstack  # noqa: E402
from concourse.bass_utils import run_bass_kernel_spmd  # noqa: E402

dt = mybir.dt

N_NODES = 50000
N_EDGES = 500000
F = 16
H = 64
HP = H + 1  # hidden + b2 row
NCORES = 8
NPC = N_NODES // NCORES  # 6250 nodes per core
NPAD = 6272  # 49 * 128
SACR = NPAD - 1  # sacrificial node row
TILE = 512
SUB = 128  # edges per segment subtile
WIN = 32  # node window per subtile
ST_ALIGN = 64  # subtiles per PSUM-bank batch


def _pack_core(dloc):
    """Greedy-pack sorted local-dst edges into 128-edge subtiles such that
    no node's run crosses a subtile boundary and each subtile spans < WIN
    nodes.  Returns (pos[e] -> padded position, b[t] window bases,
    Tn{node->subtile})."""
    ne = len(dloc)
    pos = np.empty(ne, np.int64)
    b = []
    Tn = {}
    t = -1
    fill = SUB  # force new subtile at first node
    base = -1
    i = 0
    while i < ne:
        n = dloc[i]
        j = i
        while j < ne and dloc[j] == n:
            j += 1
        d = j - i
        assert d <= SUB, f"degree {d} exceeds subtile size"
        if (SUB - fill) < d or (n - base) >= WIN:
            t += 1
            fill = 0
            base = n
            b.append(n)
        Tn[n] = t
        pos[i:j] = t * SUB + fill + np.arange(d)
        fill += d
        i = j
    return pos, np.array(b, np.int64), Tn


def _host_prep(x, edge_feat, src, dst, W1, b1, W2, b2, bias):
    x = np.asarray(x, np.float32)
    edge_feat = np.asarray(edge_feat, np.float32)
    src = np.asarray(src, np.int64)
    dst = np.asarray(dst, np.int64)
    W1 = np.asarray(W1, np.float32)
    b1 = np.asarray(b1, np.float32)
    W2 = np.asarray(W2, np.float32)
    b2 = np.asarray(b2, np.float32)
    bias = np.asarray(bias, np.float32)

    order = np.argsort(dst, kind="stable")
    dst_s = dst[order]
    src_s = src[order]
    bounds = np.searchsorted(dst_s, np.arange(NCORES + 1) * NPC)

    packs = []
    st_max = 0
    for c in range(NCORES):
        lo, hi = bounds[c], bounds[c + 1]
        dloc = dst_s[lo:hi] - c * NPC
        pos, b, Tn = _pack_core(dloc)
        packs.append((pos, b, Tn))
        st_max = max(st_max, len(b))
    st = int(np.ceil(st_max / ST_ALIGN) * ST_ALIGN)
    ep = st * SUB

    # shared constants
    W1z = np.concatenate([W1, np.zeros((F, 1), np.float32)], axis=1)  # [16,65]
    biasv = np.concatenate([b1, np.ones(1, np.float32)])[:, None]  # [65,1]
    W2p = np.concatenate([W2, b2[None, :]], axis=0)  # [65,256]
    Ired = np.zeros((128, 2, F), np.float32)
    for c2 in range(2):
        for m in range(128):
            Ired[m, c2, (c2 * 128 + m) % F] = 1.0
    Ired = Ired.astype(ml_dtypes.bfloat16)
    iota64 = np.broadcast_to(
        np.arange(WIN, dtype=np.float32), (128, WIN)
    ).copy()

    cnt_all = np.bincount(dst_s, minlength=N_NODES).astype(np.float32)

    in_maps = []
    for c in range(NCORES):
        lo, hi = bounds[c], bounds[c + 1]
        ec = hi - lo
        pos, b, Tn = packs[c]
        nst = len(b)

        efdt_np = (
            ml_dtypes.bfloat16
            if "bf16mm1"
            in os.environ.get("BASS_GNN_PARTS", "all,gb8,bf16mm1").split(",")
            else np.float32
        )
        efT = np.zeros((F, ep), efdt_np)
        xsTb = np.zeros((F, ep), ml_dtypes.bfloat16)
        efT[:, pos] = edge_feat[order[lo:hi]].T.astype(efdt_np)
        xsTb[:, pos] = x[src_s[lo:hi]].T.astype(ml_dtypes.bfloat16)

        dloc = dst_s[lo:hi] - c * NPC
        dlr = np.full((ep,), -1.0, np.float32)
        dlr[pos] = (dloc - b[(pos // SUB)]).astype(np.float32)
        assert dlr.max() < WIN
        dlocrel = dlr.reshape(st, SUB).T.copy()  # [128, st] f32

        # scatter-entry position for (t, w): the PSUM bank/slab2 layout puts
        # subtile t at partition 64*(t%2)+w, col-group (t//2)%32, bank t//64
        idx2 = np.full((st * 64,), SACR, np.int64)
        for n, t in Tn.items():
            w = n - b[t]
            i = (t // 64) * 4096 + ((t // 2) % 32) * 128 + 64 * (t % 2) + w
            idx2[i] = n
        n2 = st * 64
        idxw = np.zeros((16, n2 // 16), np.int16)
        idxw[np.arange(n2) % 16, np.arange(n2) // 16] = idx2.astype(np.int16)
        idxw = np.tile(idxw, (8, 1))

        cnt = np.zeros((NPAD,), np.float32)
        cnt[:NPC] = cnt_all[c * NPC : (c + 1) * NPC]
        A = np.where(cnt > 0, 1.0 / np.maximum(cnt, 1.0), 0.0).astype(np.float32)
        B = np.broadcast_to(bias, (NPAD, F)).copy().astype(np.float32)
        nomsg = cnt[:NPC] == 0
        if nomsg.any():
            xn = x[c * NPC : (c + 1) * NPC]
            B[:NPC][nomsg] = xn[nomsg] + bias

        in_maps.append(
            {
                "efT": efT,
                "xsTb": xsTb,
                "dlocrel": dlocrel,
                "idx2": idxw,
                "A": A[:, None],
                "B": B,
                "W1z": W1z.astype(efdt_np),
                "biasv": biasv,
                "W2p": W2p,
                "Ired": Ired,
                "iota64": iota64,
            }
        )
    return in_maps, ep, bounds


@with_exitstack
def _build_tile_kernel(ctx, tc: tile.TileContext, ep: int, parts: str = "all"):
    has = lambda p: ("all" in parts.split(",") and p in ("main","msg","seg","scat","fin")) or p in parts.split(",")
    rep = 1
    for tok in parts.split(","):
        if tok.startswith("rep"):
            rep = int(tok[3:])
    nc = tc.nc
    st = ep // SUB
    mmdt = dt.float32r if "f32r" in parts.split(",") else dt.float32
    efdt = dt.bfloat16 if "bf16mm1" in parts.split(",") else mmdt
    t_efT = nc.dram_tensor("efT", [F, ep], efdt, kind="ExternalInput").ap()
    t_xsTb = nc.dram_tensor("xsTb", [F, ep], dt.bfloat16, kind="ExternalInput").ap()
    t_dlr = nc.dram_tensor(
        "dlocrel", [128, st], dt.float32, kind="ExternalInput"
    ).ap()
    t_idx2 = nc.dram_tensor(
        "idx2", [128, (st * 64) // 16], dt.int16, kind="ExternalInput"
    ).ap()
    t_A = nc.dram_tensor("A", [NPAD, 1], dt.float32, kind="ExternalInput").ap()
    t_B = nc.dram_tensor("B", [NPAD, F], dt.float32, kind="ExternalInput").ap()
    t_W1z = nc.dram_tensor("W1z", [F, HP], efdt, kind="ExternalInput").ap()
    t_biasv = nc.dram_tensor("biasv", [HP, 1], dt.float32, kind="ExternalInput").ap()
    t_W2p = nc.dram_tensor("W2p", [HP, 256], mmdt, kind="ExternalInput").ap()
    t_Ired = nc.dram_tensor(
        "Ired", [128, 2, F], dt.bfloat16, kind="ExternalInput"
    ).ap()
    t_iota = nc.dram_tensor(
        "iota64", [128, WIN], dt.float32, kind="ExternalInput"
    ).ap()
    t_out = nc.dram_tensor("OUT", [NPAD, F], dt.float32, kind="ExternalOutput").ap()
    nb = nc.dram_tensor("node_buf", [NPAD, 64], dt.float32).ap()

    const = ctx.enter_context(tc.tile_pool(name="const", bufs=1))
    deep = 1 if "deep" in parts.split(",") else 0
    gb8 = "gb8" in parts.split(",")
    ebuf = ctx.enter_context(tc.tile_pool(name="ebuf", bufs=2 if gb8 else 3 + deep))
    hbuf = ctx.enter_context(tc.tile_pool(name="hbuf", bufs=2 + deep))
    pbuf = ctx.enter_context(tc.tile_pool(name="pbuf", bufs=3 if gb8 else 4 + 2 * deep))
    mbuf = ctx.enter_context(tc.tile_pool(name="mbuf", bufs=3 + deep))
    segb = ctx.enter_context(tc.tile_pool(name="segb", bufs=6))
    slab2 = ctx.enter_context(tc.tile_pool(name="slab2", bufs=1))
    fin = ctx.enter_context(tc.tile_pool(name="fin", bufs=3))
    ps_h = ctx.enter_context(tc.tile_pool(name="ps_h", bufs=2, space="PSUM"))
    ps_w = ctx.enter_context(tc.tile_pool(name="ps_w", bufs=2, space="PSUM"))
    ps_m = ctx.enter_context(tc.tile_pool(name="ps_m", bufs=2, space="PSUM"))
    ps_s = ctx.enter_context(tc.tile_pool(name="ps_s", bufs=2, space="PSUM"))

    nc.gpsimd.load_library(library_config.mlp)

    # constants
    w1z = const.tile([F, HP], efdt)
    nc.sync.dma_start(w1z[:], t_W1z)
    bv = const.tile([HP, 1], dt.float32)
    nc.sync.dma_start(bv[:], t_biasv)
    w2p = const.tile([HP, 256], mmdt)
    nc.sync.dma_start(w2p[:], t_W2p)
    ired = const.tile([128, 2, F], dt.bfloat16)
    nc.sync.dma_start(ired[:], t_Ired)
    iota = const.tile([128, WIN], dt.float32)
    nc.sync.dma_start(iota[:], t_iota)
    dlrt = const.tile([128, st], dt.float32)
    nc.sync.dma_start(dlrt[:], t_dlr)
    idxt = const.tile([128, (st * 64) // 16], dt.int16)
    nc.sync.dma_start(idxt[:], t_idx2)
    sl2 = slab2.tile([128, st // 2, F], dt.float32)
    nc.vector.memset(sl2[:], 0.0)

    # zero the node accumulator (batched: 1024 rows per DMA)
    JB = max(j for j in range(1, 8) if NPAD % (128 * j) == 0)
    zt = const.tile([128, JB, 64], dt.float32)
    nc.vector.memset(zt[:], 0.0)

    rep_cm = tc.For_i(0, rep, 1, name="rep") if rep > 1 else None
    if rep_cm is not None:
        ctx.enter_context(rep_cm)

    nbz = nb.rearrange("(t j p) c -> t p j c", p=128, j=JB)
    for t in range(NPAD // (128 * JB)):
        nc.sync.dma_start(nbz[t], zt[:])

    GB = 8 if "gb8" in parts.split(",") else 4  # tiles per DMA group
    nt = ep // TILE if has("main") else 0
    assert nt % GB == 0 or nt == 0
    for g in range(nt // GB):
        gs = slice(g * GB * TILE, (g + 1) * GB * TILE)
        eft4 = ebuf.tile([F, GB, TILE], efdt, tag="eft")
        nc.sync.dma_start(eft4[:], t_efT[:, gs].rearrange("p (a e) -> p a e", e=TILE))
        xsb4 = ebuf.tile([F, GB * TILE], dt.bfloat16, tag="xsb")
        nc.sync.dma_start(xsb4[:], t_xsTb[:, gs])
        xr4 = [None, None]
        for c in range(2):
            xr4[c] = pbuf.tile([128, GB, TILE], dt.bfloat16, tag=f"xr{c}", name=f"xr{c}")
            nc.sync.dma_start(
                xr4[c][:].rearrange("p a e -> p (a e)"),
                xsb4[c * 8 : (c + 1) * 8, :]
                .rearrange("p (one e) -> p one e", one=1)
                .broadcast_to([8, 16, GB * TILE]),
            )
        mt4 = mbuf.tile([F, GB, TILE], dt.bfloat16, tag="msgT")

        for tt in range(GB):
            t = g * GB + tt
            hpre = ps_h.tile([HP, TILE], dt.float32)
            nc.tensor.matmul(
                hpre[:], w1z[:], eft4[:, tt, :], start=True, stop=True
            )
            hp = hbuf.tile([HP, TILE], mmdt, tag="hp")
            nc.scalar.activation(
                hp[:], hpre[:], mybir.ActivationFunctionType.Relu, bias=bv[:]
            )

            mg = ps_m.tile([F, TILE], dt.float32, tag="mg")
            for c in range(2):
                wt = ps_w.tile([128, TILE], dt.float32, tag="wt")
                nc.tensor.matmul(
                    wt[:], w2p[:, c * 128 : (c + 1) * 128], hp[:],
                    start=True, stop=True,
                )
                pr = pbuf.tile([128, TILE], dt.bfloat16, tag="prod")
                nc.vector.tensor_tensor(
                    pr[:], wt[:], xr4[c][:, tt, :], mybir.AluOpType.mult
                )
                nc.tensor.matmul(
                    mg[:], ired[:, c, :], pr[:], start=(c == 0), stop=(c == 1)
                )

            if has("msg"):
                nc.scalar.activation(
                    mt4[:, tt, :], mg[:], mybir.ActivationFunctionType.Copy
                )

        if not has("msg"):
            continue
        tr4 = mbuf.tile([128, GB * (TILE // SUB), F], dt.bfloat16, tag="tr")
        nc.sync.dma_start(
            tr4[:], mt4[:].rearrange("p a e -> p (a e)"), transpose=True
        )

        if not has("seg"):
            continue
        for q in range(GB * (TILE // SUB)):
            stile = g * GB * (TILE // SUB) + q
            seg = segb.tile([128, WIN], dt.bfloat16, tag="seg")
            nc.gpsimd.tensor_scalar(
                seg[:],
                iota[:],
                dlrt[:, stile : stile + 1],
                None,
                mybir.AluOpType.is_equal,
            )
            if stile % ST_ALIGN == 0:
                bank = ps_s.tile([128, 512], dt.float32, tag="bank")
            prow = 64 * (stile % 2)
            pcol = ((stile // 2) % (ST_ALIGN // 2)) * F
            nc.tensor.matmul(
                bank[prow : prow + WIN, pcol : pcol + F],
                seg[:],
                tr4[:, q, :],
                start=True,
                stop=True,
            )
            if stile % ST_ALIGN == ST_ALIGN - 1:
                gg = stile // ST_ALIGN
                gsl = slice(gg * (ST_ALIGN // 2), (gg + 1) * (ST_ALIGN // 2))
                for half in range(2):
                    nc.vector.tensor_copy(
                        sl2[64 * half : 64 * half + WIN, gsl, :],
                        bank[64 * half : 64 * half + WIN, :].rearrange(
                            "p (a b) -> p a b", b=F
                        ),
                    )

    # scatter in ring-sized chunks (disjoint real rows -> safe unordered)
    csz = 4096
    for o in range(0, st * 64 if has("scat") else 0, csz):
        n_i = min(csz, st * 64 - o)
        nc.gpsimd.dma_scatter_add(
            nb[:, 0:F],
            sl2[:, o // 128 : (o + n_i) // 128, :],
            idxt[:, o // 16 : (o + n_i) // 16],
            n_i,
            n_i,
            F,
            elem_step=64,
        )

    # finalize: out = msum * A + B  (batched 512 rows per step)
    FB = 4
    nfin = NPAD // 128 if has("fin") else 0
    n = 0
    while n < nfin:
        k = min(FB, nfin - n)
        r = slice(n * 128, (n + k) * 128)
        nbt = fin.tile([128, FB, F], dt.float32, tag="nbt")
        nc.sync.dma_start(
            nbt[:, :k, :], nb[r, 0:F].rearrange("(j p) c -> p j c", p=128)
        )
        at = fin.tile([128, FB, 1], dt.float32, tag="at")
        nc.sync.dma_start(at[:, :k, :], t_A[r].rearrange("(j p) c -> p j c", p=128))
        bt = fin.tile([128, FB, F], dt.float32, tag="bt")
        nc.sync.dma_start(bt[:, :k, :], t_B[r].rearrange("(j p) c -> p j c", p=128))
        ot = fin.tile([128, FB, F], dt.float32, tag="ot")
        abc = at[:, :k, :].broadcast_to([128, k, F])
        nc.vector.tensor_tensor(ot[:, :k, :], nbt[:, :k, :], abc,
                                mybir.AluOpType.mult)
        nc.vector.tensor_tensor(ot[:, :k, :], ot[:, :k, :], bt[:, :k, :],
                                mybir.AluOpType.add)
        nc.sync.dma_start(t_out[r].rearrange("(j p) c -> p j c", p=128), ot[:, :k, :])
        n += k


def _pack_core2(dloc):
    """Bin-pack node edge-runs into 128-edge subtiles (<=WIN nodes each)
    with first-fit-decreasing over integer degree buckets. Node->slot
    assignment is free-form (the one-hot seg matrices and scatter indices
    are host-built), so no window-locality constraint applies.

    Returns (pos[e] -> packed position, slot_of_node {n: (t, w)})."""
    ne = len(dloc)
    nodes, starts, degs = np.unique(dloc, return_index=True, return_counts=True)
    assert degs.max() <= SUB
    maxd = int(degs.max())
    # degree buckets: list of node indices per degree
    buckets = [[] for _ in range(maxd + 1)]
    for ni, d in enumerate(degs):
        buckets[d].append(ni)
    remaining = len(nodes)
    bins = []
    while remaining:
        # new bin: repeatedly take the largest-degree node that still fits
        rem, slots, members = SUB, WIN, []
        while slots:
            d = min(rem, maxd)
            while d > 0 and not buckets[d]:
                d -= 1
            if d == 0:
                break
            members.append(buckets[d].pop())
            rem -= d
            slots -= 1
            remaining -= 1
        bins.append(members)
    pos = np.empty(ne, np.int64)
    slot_of_node = {}
    for t, members in enumerate(bins):
        fill = 0
        for w, ni in enumerate(members):
            d = degs[ni]
            s0 = starts[ni]
            pos[s0 : s0 + d] = t * SUB + fill
            pos[s0 : s0 + d] += np.arange(d)
            fill += d
            slot_of_node[nodes[ni]] = (t, w)
    return pos, slot_of_node, len(bins)


def _host_prep2(x, edge_feat, src, dst, W1, b1, W2, b2, bias):
    """Host prep for the edge-major v2 pipeline.

    v2 layout notes:
      - W2 columns are permuted to (o, i) order (c = o*16 + i) so the
        per-edge i-reduction is over the innermost axis.
      - xs is shipped edge-major: partition p of subtile s holds
        x[src[edge s*128+p]][:] as 16 contiguous bf16 values.
      - Z/scatter slot mapping: subtile t window-slot w sits at scatter row
        j = (t//128)*4096 + ((t//4)%32)*128 + 32*(t%4) + w, matching a
        [128, st//4, 16] SBUF accumulator with 4 subtiles per 128
        partitions and one 16-wide column group per 512-edge tile.
    """
    x = np.asarray(x, np.float32)
    edge_feat = np.asarray(edge_feat, np.float32)
    src = np.asarray(src, np.int64)
    dst = np.asarray(dst, np.int64)
    W1 = np.asarray(W1, np.float32)
    b1 = np.asarray(b1, np.float32)
    W2 = np.asarray(W2, np.float32)
    b2 = np.asarray(b2, np.float32)
    bias = np.asarray(bias, np.float32)

    order = np.argsort(dst, kind="stable")
    dst_s = dst[order]
    src_s = src[order]
    # equal-edge sharding with node-aligned boundaries
    cum = np.cumsum(np.bincount(dst_s, minlength=N_NODES))
    nbounds = np.concatenate(
        [
            [0],
            np.searchsorted(cum, (np.arange(1, NCORES) * len(dst_s)) // NCORES),
            [N_NODES],
        ]
    ).astype(np.int64)
    bounds = np.searchsorted(dst_s, nbounds)

    packs = []
    st_max = 0
    for c in range(NCORES):
        lo, hi = bounds[c], bounds[c + 1]
        dloc = dst_s[lo:hi] - nbounds[c]
        pos, slot_of_node, nst = _pack_core2(dloc)
        packs.append((pos, slot_of_node))
        st_max = max(st_max, nst)
    st = int(np.ceil(st_max / 8) * 8)
    ep = st * SUB

    # shared constants
    W2p = np.concatenate([W2, b2[None, :]], axis=0)  # [65,256] cols (i,o)
    # permute cols to (o, i): new col o*16+i <- old col i*16+o
    oc = np.arange(256)
    W2perm = W2p[:, (oc % 16) * 16 + (oc // 16)].astype(ml_dtypes.bfloat16)

    # per-edge hidden activations computed host-side (first MLP layer)
    h_all = np.maximum(edge_feat @ W1 + b1, 0.0).astype(ml_dtypes.bfloat16)

    cnt_all = np.bincount(dst_s, minlength=N_NODES).astype(np.float32)

    in_maps = []
    finals = []
    for c in range(NCORES):
        lo, hi = bounds[c], bounds[c + 1]
        pos, slot_of_node = packs[c]

        # hp feature-major [65, ep]: rows 0..63 = h.T, row 64 = ones
        hpT = np.zeros((HP, ep), ml_dtypes.bfloat16)
        hpT[:H, pos] = h_all[order[lo:hi]].T
        hpT[H, :] = 1.0

        xs = np.zeros((ep, F), ml_dtypes.bfloat16)
        xs[pos] = x[src_s[lo:hi]].astype(ml_dtypes.bfloat16)
        # [128, st*16]: partition p, free s*16+i  <-  edge s*128+p feature i
        xs_em = np.ascontiguousarray(
            xs.reshape(st, SUB, F).transpose(1, 0, 2).reshape(SUB, st * F)
        )

        # one-hot seg matrices from the free-form slot assignment
        dloc = dst_s[lo:hi] - nbounds[c]
        wof = np.array(
            [slot_of_node[n][1] for n in dloc], dtype=np.int64
        )
        segf = np.zeros((ep, WIN), ml_dtypes.bfloat16)
        segf[pos, wof] = 1.0
        seg_all = np.ascontiguousarray(
            segf.reshape(st, SUB, WIN).transpose(1, 0, 2).reshape(SUB, st * WIN)
        )

        # host-side finalize data: sl2 slot of each local node + A, B
        # sl2 layout: partition 32*(t%4)+w, column t//4
        nlo, nhi = nbounds[c], nbounds[c + 1]
        nloc = nhi - nlo
        slot_row = np.zeros((nloc,), np.int64)
        slot_col = np.zeros((nloc,), np.int64)
        for n, (t, w) in slot_of_node.items():
            slot_row[n] = 32 * (t % 4) + w
            slot_col[n] = t // 4
        cnt = cnt_all[nlo:nhi]
        A = np.where(cnt > 0, 1.0 / np.maximum(cnt, 1.0), 0.0).astype(np.float32)
        B = np.broadcast_to(bias, (nloc, F)).copy().astype(np.float32)
        nomsg = cnt == 0
        if nomsg.any():
            xn = x[nlo:nhi]
            B[nomsg] = xn[nomsg] + bias

        in_maps.append(
            {
                "hpT": hpT,
                "xs_em": xs_em,
                "seg_all": seg_all,
                "W2p": W2perm,
            }
        )
        finals.append((slot_row, slot_col, A[:, None], B))
    return in_maps, ep, nbounds, finals


@with_exitstack
def _build_tile_kernel2(ctx, tc: tile.TileContext, ep: int, parts: str = "v2"):
    """Edge-major pipeline: per 512-edge tile (4 x 128-edge subtiles)
      mm1:  hpre[65,512] = W1z.T @ efT            (PE, bf16, PSUM)
      relu: hp = relu(hpre + [b1;1]) -> bf16 SBUF (ACT)
      mm2:  wt[128e, 256] = hp[:,q].T @ W2perm    (PE x4, PSUM, cols (o,i))
      prod: pr = wt * xs_bcast  -> bf16 SBUF      (DVE o<OD, GPSIMD o>=OD)
      seg:  one-hot seg[128,32] (GPSIMD is_equal)
      segmm: z[32q:32q+32, 256] = seg.T @ pr      (PE x4, PSUM)
      red:  sl2[:, tile, :] = sum_i z             (DVE tensor_reduce X)
      scatter + finalize as v1.
    """
    toks = parts.split(",")
    PQ = 1  # product quarters on GPSIMD (reads SBUF wts), rest on DVE
    XSP = 240  # wt-drain split point: ACT copies cols [0,XSP), DVE the rest
    POSP = 0  # extra o-slices of the last DVE quarter shifted to GPSIMD
    for tk in toks:
        if tk.startswith("pq"):
            PQ = int(tk[2:])
        if tk.startswith("xsp"):
            XSP = int(tk[3:])
        if tk.startswith("po"):
            POSP = int(tk[2:])
    nc = tc.nc
    st = ep // SUB
    nt = ep // TILE  # 512-edge tiles
    GB = 8
    for tk in toks:
        if tk.startswith("gb"):
            GB = int(tk[2:])
    assert nt % 2 == 0
    # group sizes: optional small first group to shorten the pipeline head
    gsizes = []
    rem = nt
    if "g02" in toks:
        gsizes.append(2)
        rem -= 2
    while rem > 0:
        gsizes.append(min(GB, rem))
        rem -= gsizes[-1]

    t_hp = nc.dram_tensor("hpT", [HP, ep], dt.bfloat16, kind="ExternalInput").ap()
    t_xs = nc.dram_tensor(
        "xs_em", [SUB, st * F], dt.bfloat16, kind="ExternalInput"
    ).ap()
    t_seg = nc.dram_tensor(
        "seg_all", [SUB, st * WIN], dt.bfloat16, kind="ExternalInput"
    ).ap()
    t_W2p = nc.dram_tensor("W2p", [HP, 256], dt.bfloat16, kind="ExternalInput").ap()
    t_out = nc.dram_tensor(
        "OUT", [128, nt * F], dt.float32, kind="ExternalOutput"
    ).ap()

    const = ctx.enter_context(tc.tile_pool(name="const", bufs=1))
    hbuf = ctx.enter_context(tc.tile_pool(name="hbuf", bufs=2))
    sgb = ctx.enter_context(tc.tile_pool(name="sgb", bufs=2))
    xsb = ctx.enter_context(tc.tile_pool(name="xsb", bufs=2))
    wtsb = ctx.enter_context(tc.tile_pool(name="wtsb", bufs=3))
    prodb = ctx.enter_context(tc.tile_pool(name="prodb", bufs=4))
    sl2p = ctx.enter_context(tc.tile_pool(name="sl2p", bufs=2))

    ps_w = ctx.enter_context(tc.tile_pool(name="ps_w", bufs=3, space="PSUM"))
    ps_z = ctx.enter_context(tc.tile_pool(name="ps_z", bufs=2, space="PSUM"))

    nc.gpsimd.load_library(library_config.mlp)

    # constants
    w2p = const.tile([HP, 256], dt.bfloat16)
    nc.sync.dma_start(w2p[:], t_W2p)

    mult = mybir.AluOpType.mult
    for g in range((nt + GB - 1) // GB):
        GBg = min(GB, nt - g * GB)
        SGC = GBg * 4  # subtiles in this group
        hp8 = hbuf.tile([HP, GBg, TILE], dt.bfloat16, tag="hp")
        nc.sync.dma_start(
            hp8[:],
            t_hp[:, g * GB * TILE : (g * GB + GBg) * TILE].rearrange(
                "p (a e) -> p a e", e=TILE
            ),
        )
        sgt = sgb.tile([128, SGC, WIN], dt.bfloat16, tag="sg")
        nc.sync.dma_start(
            sgt[:].rearrange("p s w -> p (s w)"),
            t_seg[:, g * GB * 4 * WIN : (g * GB * 4 + SGC) * WIN],
        )
        xst = xsb.tile([128, SGC, F], dt.bfloat16, tag="xs")
        nc.sync.dma_start(
            xst[:].rearrange("p s i -> p (s i)"),
            t_xs[:, g * GB * 4 * F : (g * GB * 4 + SGC) * F],
        )
        sl2 = sl2p.tile([128, GBg, F], dt.float32, tag="sl2")
        for tt in range(GBg):
            ti = g * GB + tt  # 512-edge tile index
            wt4 = ps_w.tile([128, 4, 256], dt.float32, tag="wt")
            for q in range(4):
                nc.tensor.matmul(
                    wt4[:, q, :],
                    hp8[:, tt, q * 128 : (q + 1) * 128],
                    w2p[:],
                    start=True,
                    stop=True,
                )
            # drain W^T to SBUF bf16 (ACT main + DVE sliver) so the product
            # runs in DVE 2x mode and GPSIMD can take a share (GPSIMD
            # cannot touch PSUM)
            wts = wtsb.tile([128, 4, 256], dt.bfloat16, tag="wts")
            nc.scalar.activation(
                wts[:, :, 0:XSP], wt4[:, :, 0:XSP],
                mybir.ActivationFunctionType.Copy,
            )
            nc.vector.tensor_copy(wts[:, :, XSP:256], wt4[:, :, XSP:256])
            pr4 = prodb.tile([128, 4, F, F], dt.bfloat16, tag="pr")
            wtv = wts[:].rearrange("p a (o i) -> p a o i", i=F)
            xsv = xst[:, 4 * tt : 4 * tt + 4, :].rearrange(
                "p a (one i) -> p a one i", one=1
            )
            DQ = 4 - PQ
            OS = F - POSP
            nc.vector.tensor_tensor(
                pr4[:, 0 : DQ - 1],
                wtv[:, 0 : DQ - 1],
                xsv[:, 0 : DQ - 1].broadcast_to([128, DQ - 1, F, F]),
                mult,
            )
            nc.vector.tensor_tensor(
                pr4[:, DQ - 1 : DQ, 0:OS],
                wtv[:, DQ - 1 : DQ, 0:OS],
                xsv[:, DQ - 1 : DQ].broadcast_to([128, 1, OS, F]),
                mult,
            )
            if POSP:
                nc.gpsimd.tensor_tensor(
                    pr4[:, DQ - 1 : DQ, OS:F],
                    wtv[:, DQ - 1 : DQ, OS:F],
                    xsv[:, DQ - 1 : DQ].broadcast_to([128, 1, POSP, F]),
                    mult,
                )
            if PQ:
                nc.gpsimd.tensor_tensor(
                    pr4[:, DQ:4],
                    wtv[:, DQ:4],
                    xsv[:, DQ:4].broadcast_to([128, PQ, F, F]),
                    mult,
                )
            if tt % 2 == 0:
                z8 = ps_z.tile([128, 2, 256], dt.float32, tag="z")
            for q in range(4):
                nc.tensor.matmul(
                    z8[32 * q : 32 * q + 32, tt % 2, :],
                    sgt[:, 4 * tt + q, :],
                    pr4[:, q, :, :].rearrange("p o i -> p (o i)"),
                    start=True,
                    stop=True,
                    tile_position=(0, 32 * q),
                )
            if tt % 2 == 1:
                nc.vector.tensor_reduce(
                    sl2[:, tt - 1 : tt + 1, :],
                    z8[:].rearrange("p a (o i) -> p a o i", i=F),
                    mybir.AxisListType.X,
                    mybir.AluOpType.add,
                )
        # stream this group's per-slot sums out; host does the node
        # permutation and the *A + B finalize
        nc.sync.dma_start(
            t_out[:, g * GB * F : (g * GB + GBg) * F],
            sl2[:].rearrange("p a c -> p (a c)"),
        )


_CACHE = {}


PARTS = os.environ.get("BASS_GNN_PARTS", "v2")


def _get_program(ep: int):
    key = (ep, PARTS)
    if key not in _CACHE:
        nc = bacc.Bacc("TRN2", target_bir_lowering=False, debug=False)
        with tile.TileContext(nc) as tc:
            if "v2" in PARTS.split(","):
                _build_tile_kernel2(tc, ep, parts=PARTS)
            else:
                _build_tile_kernel(tc, ep, parts=PARTS)
        nc.compile()
        _CACHE[key] = nc
    return _CACHE[key]


LAST_RESULTS = None


def kernel(x, edge_feat, src, dst, W1, b1, W2, b2, bias):
    global LAST_RESULTS
    v2 = "v2" in PARTS.split(",")
    if v2:
        in_maps, ep, nbounds, finals = _host_prep2(
            x, edge_feat, src, dst, W1, b1, W2, b2, bias
        )
    else:
        in_maps, ep, _ = _host_prep(x, edge_feat, src, dst, W1, b1, W2, b2, bias)
    nc = _get_program(ep)
    trace = os.environ.get("BASS_GNN_TRACE", "") == "1"
    res = run_bass_kernel_spmd(nc, in_maps, list(range(NCORES)), trace=trace)
    LAST_RESULTS = res
    out = np.empty((N_NODES, F), np.float32)
    for c in range(NCORES):
        o = res.results[c]["OUT"]
        if v2:
            nt = ep // TILE
            slot_row, slot_col, A, B = finals[c]
            sl2 = o.reshape(128, nt, F)
            msum = sl2[slot_row, slot_col]
            out[nbounds[c] : nbounds[c + 1]] = msum * A + B
        else:
            out[c * NPC : (c + 1) * NPC] = o[:NPC]
    return out



# revision 63
# speedup vs baseline: 1.0294x; 1.0294x over previous
"""DynamicGraphConv Trainium2 kernel (8 NeuronCores, SPMD).

Edges are sorted by destination and sharded across cores at node-aligned
equal-edge boundaries, so each core owns a disjoint slice of the output
and no cross-core collective is needed.

Host prep (index bookkeeping + first MLP layer):
  - h = relu(edge_feat @ W1 + b1) computed host-side, shipped as
    hp^T [65, ep] bf16 (row 64 = ones, folding b2 via W2p's extra row).
  - x[src] gathered per-edge, shipped edge-major ([128, st, 16] bf16).
  - Edge runs (one per destination node) are bin-packed (FFD over degree
    buckets) into 128-edge subtiles, <=32 node-slots each; the one-hot
    seg matrices [128, st, 32] bf16 are built host-side.
  - W2 columns permuted to (o, i) order; b2 appended as row 64.

Per-core device pipeline (v2, edge-major, 512-edge tiles = 4 subtiles):
  mm2:   wt[128e, 256] = hp[:,q].T @ W2p          (PE x4, PSUM fp32)
  drain: wts = bf16(wt)                           (ACT cols<240, DVE rest)
  prod:  pr[e,(o,i)] = wts * xs[e,i]              (DVE 2x q0-2, GPSIMD q3;
         xs broadcast along o via stride-0 AP — no replication DMA)
  segmm: z[32q:32q+32, 256] = seg_q.T @ pr_q      (PE x4, one PSUM bank
         per 2 tiles, tile_position column offsets)
  red:   sl2[:, tile, :] = sum_i z                (DVE tensor_reduce X,
         node-level: 10x fewer rows than edges)
  out:   sl2 [128, 8, 16] streamed to DRAM per group.

Host finalize: out[n] = sl2[slot(n)] * A[n] + B[n] (pure permutation;
A = 1/max(cnt,1) masked, B = bias or x[n]+bias).
"""

import os
import sys

import numpy as np

for _p in ("/opt/trn_rl_repo",):
    if _p not in sys.path and os.path.isdir(_p):
        sys.path.insert(0, _p)

import ml_dtypes  # noqa: E402

import concourse.bass as bass  # noqa: E402
import concourse.tile as tile  # noqa: E402
from concourse import bacc  # noqa: E402
from concourse import library_config  # noqa: E402
from concourse import mybir  # noqa: E402
from concourse._compat import with_exitstack  # noqa: E402
from concourse.bass_utils import run_bass_kernel_spmd  # noqa: E402

dt = mybir.dt

N_NODES = 50000
N_EDGES = 500000
F = 16
H = 64
HP = H + 1  # hidden + b2 row
NCORES = 8
NPC = N_NODES // NCORES  # 6250 nodes per core
NPAD = 6272  # 49 * 128
SACR = NPAD - 1  # sacrificial node row
TILE = 512
SUB = 128  # edges per segment subtile
WIN = 32  # node window per subtile
ST_ALIGN = 64  # subtiles per PSUM-bank batch


def _pack_core(dloc):
    """Greedy-pack sorted local-dst edges into 128-edge subtiles such that
    no node's run crosses a subtile boundary and each subtile spans < WIN
    nodes.  Returns (pos[e] -> padded position, b[t] window bases,
    Tn{node->subtile})."""
    ne = len(dloc)
    pos = np.empty(ne, np.int64)
    b = []
    Tn = {}
    t = -1
    fill = SUB  # force new subtile at first node
    base = -1
    i = 0
    while i < ne:
        n = dloc[i]
        j = i
        while j < ne and dloc[j] == n:
            j += 1
        d = j - i
        assert d <= SUB, f"degree {d} exceeds subtile size"
        if (SUB - fill) < d or (n - base) >= WIN:
            t += 1
            fill = 0
            base = n
            b.append(n)
        Tn[n] = t
        pos[i:j] = t * SUB + fill + np.arange(d)
        fill += d
        i = j
    return pos, np.array(b, np.int64), Tn


def _host_prep(x, edge_feat, src, dst, W1, b1, W2, b2, bias):
    x = np.asarray(x, np.float32)
    edge_feat = np.asarray(edge_feat, np.float32)
    src = np.asarray(src, np.int64)
    dst = np.asarray(dst, np.int64)
    W1 = np.asarray(W1, np.float32)
    b1 = np.asarray(b1, np.float32)
    W2 = np.asarray(W2, np.float32)
    b2 = np.asarray(b2, np.float32)
    bias = np.asarray(bias, np.float32)

    order = np.argsort(dst, kind="stable")
    dst_s = dst[order]
    src_s = src[order]
    bounds = np.searchsorted(dst_s, np.arange(NCORES + 1) * NPC)

    packs = []
    st_max = 0
    for c in range(NCORES):
        lo, hi = bounds[c], bounds[c + 1]
        dloc = dst_s[lo:hi] - c * NPC
        pos, b, Tn = _pack_core(dloc)
        packs.append((pos, b, Tn))
        st_max = max(st_max, len(b))
    st = int(np.ceil(st_max / ST_ALIGN) * ST_ALIGN)
    ep = st * SUB

    # shared constants
    W1z = np.concatenate([W1, np.zeros((F, 1), np.float32)], axis=1)  # [16,65]
    biasv = np.concatenate([b1, np.ones(1, np.float32)])[:, None]  # [65,1]
    W2p = np.concatenate([W2, b2[None, :]], axis=0)  # [65,256]
    Ired = np.zeros((128, 2, F), np.float32)
    for c2 in range(2):
        for m in range(128):
            Ired[m, c2, (c2 * 128 + m) % F] = 1.0
    Ired = Ired.astype(ml_dtypes.bfloat16)
    iota64 = np.broadcast_to(
        np.arange(WIN, dtype=np.float32), (128, WIN)
    ).copy()

    cnt_all = np.bincount(dst_s, minlength=N_NODES).astype(np.float32)

    in_maps = []
    for c in range(NCORES):
        lo, hi = bounds[c], bounds[c + 1]
        ec = hi - lo
        pos, b, Tn = packs[c]
        nst = len(b)

        efdt_np = (
            ml_dtypes.bfloat16
            if "bf16mm1"
            in os.environ.get("BASS_GNN_PARTS", "all,gb8,bf16mm1").split(",")
            else np.float32
        )
        efT = np.zeros((F, ep), efdt_np)
        xsTb = np.zeros((F, ep), ml_dtypes.bfloat16)
        efT[:, pos] = edge_feat[order[lo:hi]].T.astype(efdt_np)
        xsTb[:, pos] = x[src_s[lo:hi]].T.astype(ml_dtypes.bfloat16)

        dloc = dst_s[lo:hi] - c * NPC
        dlr = np.full((ep,), -1.0, np.float32)
        dlr[pos] = (dloc - b[(pos // SUB)]).astype(np.float32)
        assert dlr.max() < WIN
        dlocrel = dlr.reshape(st, SUB).T.copy()  # [128, st] f32

        # scatter-entry position for (t, w): the PSUM bank/slab2 layout puts
        # subtile t at partition 64*(t%2)+w, col-group (t//2)%32, bank t//64
        idx2 = np.full((st * 64,), SACR, np.int64)
        for n, t in Tn.items():
            w = n - b[t]
            i = (t // 64) * 4096 + ((t // 2) % 32) * 128 + 64 * (t % 2) + w
            idx2[i] = n
        n2 = st * 64
        idxw = np.zeros((16, n2 // 16), np.int16)
        idxw[np.arange(n2) % 16, np.arange(n2) // 16] = idx2.astype(np.int16)
        idxw = np.tile(idxw, (8, 1))

        cnt = np.zeros((NPAD,), np.float32)
        cnt[:NPC] = cnt_all[c * NPC : (c + 1) * NPC]
        A = np.where(cnt > 0, 1.0 / np.maximum(cnt, 1.0), 0.0).astype(np.float32)
        B = np.broadcast_to(bias, (NPAD, F)).copy().astype(np.float32)
        nomsg = cnt[:NPC] == 0
        if nomsg.any():
            xn = x[c * NPC : (c + 1) * NPC]
            B[:NPC][nomsg] = xn[nomsg] + bias

        in_maps.append(
            {
                "efT": efT,
                "xsTb": xsTb,
                "dlocrel": dlocrel,
                "idx2": idxw,
                "A": A[:, None],
                "B": B,
                "W1z": W1z.astype(efdt_np),
                "biasv": biasv,
                "W2p": W2p,
                "Ired": Ired,
                "iota64": iota64,
            }
        )
    return in_maps, ep, bounds


@with_exitstack
def _build_tile_kernel(ctx, tc: tile.TileContext, ep: int, parts: str = "all"):
    has = lambda p: ("all" in parts.split(",") and p in ("main","msg","seg","scat","fin")) or p in parts.split(",")
    rep = 1
    for tok in parts.split(","):
        if tok.startswith("rep"):
            rep = int(tok[3:])
    nc = tc.nc
    st = ep // SUB
    mmdt = dt.float32r if "f32r" in parts.split(",") else dt.float32
    efdt = dt.bfloat16 if "bf16mm1" in parts.split(",") else mmdt
    t_efT = nc.dram_tensor("efT", [F, ep], efdt, kind="ExternalInput").ap()
    t_xsTb = nc.dram_tensor("xsTb", [F, ep], dt.bfloat16, kind="ExternalInput").ap()
    t_dlr = nc.dram_tensor(
        "dlocrel", [128, st], dt.float32, kind="ExternalInput"
    ).ap()
    t_idx2 = nc.dram_tensor(
        "idx2", [128, (st * 64) // 16], dt.int16, kind="ExternalInput"
    ).ap()
    t_A = nc.dram_tensor("A", [NPAD, 1], dt.float32, kind="ExternalInput").ap()
    t_B = nc.dram_tensor("B", [NPAD, F], dt.float32, kind="ExternalInput").ap()
    t_W1z = nc.dram_tensor("W1z", [F, HP], efdt, kind="ExternalInput").ap()
    t_biasv = nc.dram_tensor("biasv", [HP, 1], dt.float32, kind="ExternalInput").ap()
    t_W2p = nc.dram_tensor("W2p", [HP, 256], mmdt, kind="ExternalInput").ap()
    t_Ired = nc.dram_tensor(
        "Ired", [128, 2, F], dt.bfloat16, kind="ExternalInput"
    ).ap()
    t_iota = nc.dram_tensor(
        "iota64", [128, WIN], dt.float32, kind="ExternalInput"
    ).ap()
    t_out = nc.dram_tensor("OUT", [NPAD, F], dt.float32, kind="ExternalOutput").ap()
    nb = nc.dram_tensor("node_buf", [NPAD, 64], dt.float32).ap()

    const = ctx.enter_context(tc.tile_pool(name="const", bufs=1))
    deep = 1 if "deep" in parts.split(",") else 0
    gb8 = "gb8" in parts.split(",")
    ebuf = ctx.enter_context(tc.tile_pool(name="ebuf", bufs=2 if gb8 else 3 + deep))
    hbuf = ctx.enter_context(tc.tile_pool(name="hbuf", bufs=2 + deep))
    pbuf = ctx.enter_context(tc.tile_pool(name="pbuf", bufs=3 if gb8 else 4 + 2 * deep))
    mbuf = ctx.enter_context(tc.tile_pool(name="mbuf", bufs=3 + deep))
    segb = ctx.enter_context(tc.tile_pool(name="segb", bufs=6))
    slab2 = ctx.enter_context(tc.tile_pool(name="slab2", bufs=1))
    fin = ctx.enter_context(tc.tile_pool(name="fin", bufs=3))
    ps_h = ctx.enter_context(tc.tile_pool(name="ps_h", bufs=2, space="PSUM"))
    ps_w = ctx.enter_context(tc.tile_pool(name="ps_w", bufs=2, space="PSUM"))
    ps_m = ctx.enter_context(tc.tile_pool(name="ps_m", bufs=2, space="PSUM"))
    ps_s = ctx.enter_context(tc.tile_pool(name="ps_s", bufs=2, space="PSUM"))

    nc.gpsimd.load_library(library_config.mlp)

    # constants
    w1z = const.tile([F, HP], efdt)
    nc.sync.dma_start(w1z[:], t_W1z)
    bv = const.tile([HP, 1], dt.float32)
    nc.sync.dma_start(bv[:], t_biasv)
    w2p = const.tile([HP, 256], mmdt)
    nc.sync.dma_start(w2p[:], t_W2p)
    ired = const.tile([128, 2, F], dt.bfloat16)
    nc.sync.dma_start(ired[:], t_Ired)
    iota = const.tile([128, WIN], dt.float32)
    nc.sync.dma_start(iota[:], t_iota)
    dlrt = const.tile([128, st], dt.float32)
    nc.sync.dma_start(dlrt[:], t_dlr)
    idxt = const.tile([128, (st * 64) // 16], dt.int16)
    nc.sync.dma_start(idxt[:], t_idx2)
    sl2 = slab2.tile([128, st // 2, F], dt.float32)
    nc.vector.memset(sl2[:], 0.0)

    # zero the node accumulator (batched: 1024 rows per DMA)
    JB = max(j for j in range(1, 8) if NPAD % (128 * j) == 0)
    zt = const.tile([128, JB, 64], dt.float32)
    nc.vector.memset(zt[:], 0.0)

    rep_cm = tc.For_i(0, rep, 1, name="rep") if rep > 1 else None
    if rep_cm is not None:
        ctx.enter_context(rep_cm)

    nbz = nb.rearrange("(t j p) c -> t p j c", p=128, j=JB)
    for t in range(NPAD // (128 * JB)):
        nc.sync.dma_start(nbz[t], zt[:])

    GB = 8 if "gb8" in parts.split(",") else 4  # tiles per DMA group
    nt = ep // TILE if has("main") else 0
    assert nt % GB == 0 or nt == 0
    for g in range(nt // GB):
        gs = slice(g * GB * TILE, (g + 1) * GB * TILE)
        eft4 = ebuf.tile([F, GB, TILE], efdt, tag="eft")
        nc.sync.dma_start(eft4[:], t_efT[:, gs].rearrange("p (a e) -> p a e", e=TILE))
        xsb4 = ebuf.tile([F, GB * TILE], dt.bfloat16, tag="xsb")
        nc.sync.dma_start(xsb4[:], t_xsTb[:, gs])
        xr4 = [None, None]
        for c in range(2):
            xr4[c] = pbuf.tile([128, GB, TILE], dt.bfloat16, tag=f"xr{c}", name=f"xr{c}")
            nc.sync.dma_start(
                xr4[c][:].rearrange("p a e -> p (a e)"),
                xsb4[c * 8 : (c + 1) * 8, :]
                .rearrange("p (one e) -> p one e", one=1)
                .broadcast_to([8, 16, GB * TILE]),
            )
        mt4 = mbuf.tile([F, GB, TILE], dt.bfloat16, tag="msgT")

        for tt in range(GB):
            t = g * GB + tt
            hpre = ps_h.tile([HP, TILE], dt.float32)
            nc.tensor.matmul(
                hpre[:], w1z[:], eft4[:, tt, :], start=True, stop=True
            )
            hp = hbuf.tile([HP, TILE], mmdt, tag="hp")
            nc.scalar.activation(
                hp[:], hpre[:], mybir.ActivationFunctionType.Relu, bias=bv[:]
            )

            mg = ps_m.tile([F, TILE], dt.float32, tag="mg")
            for c in range(2):
                wt = ps_w.tile([128, TILE], dt.float32, tag="wt")
                nc.tensor.matmul(
                    wt[:], w2p[:, c * 128 : (c + 1) * 128], hp[:],
                    start=True, stop=True,
                )
                pr = pbuf.tile([128, TILE], dt.bfloat16, tag="prod")
                nc.vector.tensor_tensor(
                    pr[:], wt[:], xr4[c][:, tt, :], mybir.AluOpType.mult
                )
                nc.tensor.matmul(
                    mg[:], ired[:, c, :], pr[:], start=(c == 0), stop=(c == 1)
                )

            if has("msg"):
                nc.scalar.activation(
                    mt4[:, tt, :], mg[:], mybir.ActivationFunctionType.Copy
                )

        if not has("msg"):
            continue
        tr4 = mbuf.tile([128, GB * (TILE // SUB), F], dt.bfloat16, tag="tr")
        nc.sync.dma_start(
            tr4[:], mt4[:].rearrange("p a e -> p (a e)"), transpose=True
        )

        if not has("seg"):
            continue
        for q in range(GB * (TILE // SUB)):
            stile = g * GB * (TILE // SUB) + q
            seg = segb.tile([128, WIN], dt.bfloat16, tag="seg")
            nc.gpsimd.tensor_scalar(
                seg[:],
                iota[:],
                dlrt[:, stile : stile + 1],
                None,
                mybir.AluOpType.is_equal,
            )
            if stile % ST_ALIGN == 0:
                bank = ps_s.tile([128, 512], dt.float32, tag="bank")
            prow = 64 * (stile % 2)
            pcol = ((stile // 2) % (ST_ALIGN // 2)) * F
            nc.tensor.matmul(
                bank[prow : prow + WIN, pcol : pcol + F],
                seg[:],
                tr4[:, q, :],
                start=True,
                stop=True,
            )
            if stile % ST_ALIGN == ST_ALIGN - 1:
                gg = stile // ST_ALIGN
                gsl = slice(gg * (ST_ALIGN // 2), (gg + 1) * (ST_ALIGN // 2))
                for half in range(2):
                    nc.vector.tensor_copy(
                        sl2[64 * half : 64 * half + WIN, gsl, :],
                        bank[64 * half : 64 * half + WIN, :].rearrange(
                            "p (a b) -> p a b", b=F
                        ),
                    )

    # scatter in ring-sized chunks (disjoint real rows -> safe unordered)
    csz = 4096
    for o in range(0, st * 64 if has("scat") else 0, csz):
        n_i = min(csz, st * 64 - o)
        nc.gpsimd.dma_scatter_add(
            nb[:, 0:F],
            sl2[:, o // 128 : (o + n_i) // 128, :],
            idxt[:, o // 16 : (o + n_i) // 16],
            n_i,
            n_i,
            F,
            elem_step=64,
        )

    # finalize: out = msum * A + B  (batched 512 rows per step)
    FB = 4
    nfin = NPAD // 128 if has("fin") else 0
    n = 0
    while n < nfin:
        k = min(FB, nfin - n)
        r = slice(n * 128, (n + k) * 128)
        nbt = fin.tile([128, FB, F], dt.float32, tag="nbt")
        nc.sync.dma_start(
            nbt[:, :k, :], nb[r, 0:F].rearrange("(j p) c -> p j c", p=128)
        )
        at = fin.tile([128, FB, 1], dt.float32, tag="at")
        nc.sync.dma_start(at[:, :k, :], t_A[r].rearrange("(j p) c -> p j c", p=128))
        bt = fin.tile([128, FB, F], dt.float32, tag="bt")
        nc.sync.dma_start(bt[:, :k, :], t_B[r].rearrange("(j p) c -> p j c", p=128))
        ot = fin.tile([128, FB, F], dt.float32, tag="ot")
        abc = at[:, :k, :].broadcast_to([128, k, F])
        nc.vector.tensor_tensor(ot[:, :k, :], nbt[:, :k, :], abc,
                                mybir.AluOpType.mult)
        nc.vector.tensor_tensor(ot[:, :k, :], ot[:, :k, :], bt[:, :k, :],
                                mybir.AluOpType.add)
        nc.sync.dma_start(t_out[r].rearrange("(j p) c -> p j c", p=128), ot[:, :k, :])
        n += k


def _pack_core2(dloc):
    """Bin-pack node edge-runs into 128-edge subtiles (<=WIN nodes each)
    with first-fit-decreasing over integer degree buckets. Node->slot
    assignment is free-form (the one-hot seg matrices and scatter indices
    are host-built), so no window-locality constraint applies.

    Returns (pos[e] -> packed position, slot_of_node {n: (t, w)})."""
    ne = len(dloc)
    nodes, starts, degs = np.unique(dloc, return_index=True, return_counts=True)
    assert degs.max() <= SUB
    maxd = int(degs.max())
    # degree buckets: list of node indices per degree
    buckets = [[] for _ in range(maxd + 1)]
    for ni, d in enumerate(degs):
        buckets[d].append(ni)
    remaining = len(nodes)
    bins = []
    while remaining:
        # new bin: repeatedly take the largest-degree node that still fits
        rem, slots, members = SUB, WIN, []
        while slots:
            d = min(rem, maxd)
            while d > 0 and not buckets[d]:
                d -= 1
            if d == 0:
                break
            members.append(buckets[d].pop())
            rem -= d
            slots -= 1
            remaining -= 1
        bins.append(members)
    pos = np.empty(ne, np.int64)
    slot_of_node = {}
    for t, members in enumerate(bins):
        fill = 0
        for w, ni in enumerate(members):
            d = degs[ni]
            s0 = starts[ni]
            pos[s0 : s0 + d] = t * SUB + fill
            pos[s0 : s0 + d] += np.arange(d)
            fill += d
            slot_of_node[nodes[ni]] = (t, w)
    return pos, slot_of_node, len(bins)


def _host_prep2(x, edge_feat, src, dst, W1, b1, W2, b2, bias):
    """Host prep for the edge-major v2 pipeline.

    v2 layout notes:
      - W2 columns are permuted to (o, i) order (c = o*16 + i) so the
        per-edge i-reduction is over the innermost axis.
      - xs is shipped edge-major: partition p of subtile s holds
        x[src[edge s*128+p]][:] as 16 contiguous bf16 values.
      - Z/scatter slot mapping: subtile t window-slot w sits at scatter row
        j = (t//128)*4096 + ((t//4)%32)*128 + 32*(t%4) + w, matching a
        [128, st//4, 16] SBUF accumulator with 4 subtiles per 128
        partitions and one 16-wide column group per 512-edge tile.
    """
    x = np.asarray(x, np.float32)
    edge_feat = np.asarray(edge_feat, np.float32)
    src = np.asarray(src, np.int64)
    dst = np.asarray(dst, np.int64)
    W1 = np.asarray(W1, np.float32)
    b1 = np.asarray(b1, np.float32)
    W2 = np.asarray(W2, np.float32)
    b2 = np.asarray(b2, np.float32)
    bias = np.asarray(bias, np.float32)

    order = np.argsort(dst, kind="stable")
    dst_s = dst[order]
    src_s = src[order]
    # equal-edge sharding with node-aligned boundaries
    cum = np.cumsum(np.bincount(dst_s, minlength=N_NODES))
    nbounds = np.concatenate(
        [
            [0],
            np.searchsorted(cum, (np.arange(1, NCORES) * len(dst_s)) // NCORES),
            [N_NODES],
        ]
    ).astype(np.int64)
    bounds = np.searchsorted(dst_s, nbounds)

    packs = []
    st_max = 0
    for c in range(NCORES):
        lo, hi = bounds[c], bounds[c + 1]
        dloc = dst_s[lo:hi] - nbounds[c]
        pos, slot_of_node, nst = _pack_core2(dloc)
        packs.append((pos, slot_of_node))
        st_max = max(st_max, nst)
    st = int(np.ceil(st_max / 8) * 8)
    ep = st * SUB

    # shared constants
    W2p = np.concatenate([W2, b2[None, :]], axis=0)  # [65,256] cols (i,o)
    # permute cols to (o, i): new col o*16+i <- old col i*16+o
    oc = np.arange(256)
    W2perm = W2p[:, (oc % 16) * 16 + (oc // 16)].astype(ml_dtypes.bfloat16)

    # per-edge hidden activations computed host-side (first MLP layer)
    h_all = np.maximum(edge_feat @ W1 + b1, 0.0).astype(ml_dtypes.bfloat16)

    cnt_all = np.bincount(dst_s, minlength=N_NODES).astype(np.float32)

    in_maps = []
    finals = []
    for c in range(NCORES):
        lo, hi = bounds[c], bounds[c + 1]
        pos, slot_of_node = packs[c]

        # hp feature-major [65, ep]: rows 0..63 = h.T, row 64 = ones
        hpT = np.zeros((HP, ep), ml_dtypes.bfloat16)
        hpT[:H, pos] = h_all[order[lo:hi]].T
        hpT[H, :] = 1.0

        xs = np.zeros((ep, F), ml_dtypes.bfloat16)
        xs[pos] = x[src_s[lo:hi]].astype(ml_dtypes.bfloat16)
        # [128, st*16]: partition p, free s*16+i  <-  edge s*128+p feature i
        xs_em = np.ascontiguousarray(
            xs.reshape(st, SUB, F).transpose(1, 0, 2).reshape(SUB, st * F)
        )

        # one-hot seg matrices from the free-form slot assignment
        dloc = dst_s[lo:hi] - nbounds[c]
        wof = np.array(
            [slot_of_node[n][1] for n in dloc], dtype=np.int64
        )
        segf = np.zeros((ep, WIN), ml_dtypes.bfloat16)
        segf[pos, wof] = 1.0
        seg_all = np.ascontiguousarray(
            segf.reshape(st, SUB, WIN).transpose(1, 0, 2).reshape(SUB, st * WIN)
        )

        # host-side finalize data: sl2 slot of each local node + A, B
        # sl2 layout: partition 32*(t%4)+w, column t//4
        nlo, nhi = nbounds[c], nbounds[c + 1]
        nloc = nhi - nlo
        slot_row = np.zeros((nloc,), np.int64)
        slot_col = np.zeros((nloc,), np.int64)
        for n, (t, w) in slot_of_node.items():
            slot_row[n] = 32 * (t % 4) + w
            slot_col[n] = t // 4
        cnt = cnt_all[nlo:nhi]
        A = np.where(cnt > 0, 1.0 / np.maximum(cnt, 1.0), 0.0).astype(np.float32)
        B = np.broadcast_to(bias, (nloc, F)).copy().astype(np.float32)
        nomsg = cnt == 0
        if nomsg.any():
            xn = x[nlo:nhi]
            B[nomsg] = xn[nomsg] + bias

        in_maps.append(
            {
                "hpT": hpT,
                "xs_em": xs_em,
                "seg_all": seg_all,
                "W2p": W2perm,
            }
        )
        finals.append((slot_row, slot_col, A[:, None], B))
    return in_maps, ep, nbounds, finals


@with_exitstack
def _build_tile_kernel2(ctx, tc: tile.TileContext, ep: int, parts: str = "v2"):
    """Edge-major pipeline: per 512-edge tile (4 x 128-edge subtiles)
      mm1:  hpre[65,512] = W1z.T @ efT            (PE, bf16, PSUM)
      relu: hp = relu(hpre + [b1;1]) -> bf16 SBUF (ACT)
      mm2:  wt[128e, 256] = hp[:,q].T @ W2perm    (PE x4, PSUM, cols (o,i))
      prod: pr = wt * xs_bcast  -> bf16 SBUF      (DVE o<OD, GPSIMD o>=OD)
      seg:  one-hot seg[128,32] (GPSIMD is_equal)
      segmm: z[32q:32q+32, 256] = seg.T @ pr      (PE x4, PSUM)
      red:  sl2[:, tile, :] = sum_i z             (DVE tensor_reduce X)
      scatter + finalize as v1.
    """
    toks = parts.split(",")
    PQ = 1  # product quarters on GPSIMD (reads SBUF wts), rest on DVE
    XSP = 236  # wt-drain split point: ACT copies cols [0,XSP), DVE the rest
    POSP = 0  # extra o-slices of the last DVE quarter shifted to GPSIMD
    for tk in toks:
        if tk.startswith("pq"):
            PQ = int(tk[2:])
        if tk.startswith("xsp"):
            XSP = int(tk[3:])
        if tk.startswith("po"):
            POSP = int(tk[2:])
    nc = tc.nc
    st = ep // SUB
    nt = ep // TILE  # 512-edge tiles
    GB = 8
    for tk in toks:
        if tk.startswith("gb"):
            GB = int(tk[2:])
    assert nt % 2 == 0
    # group sizes: small first group shortens the pipeline head
    gsizes = []
    rem = nt
    if nt > GB and "nog02" not in toks:
        gsizes.append(2)
        rem -= 2
    while rem > 0:
        gsizes.append(min(GB, rem))
        rem -= gsizes[-1]

    t_hp = nc.dram_tensor("hpT", [HP, ep], dt.bfloat16, kind="ExternalInput").ap()
    t_xs = nc.dram_tensor(
        "xs_em", [SUB, st * F], dt.bfloat16, kind="ExternalInput"
    ).ap()
    t_seg = nc.dram_tensor(
        "seg_all", [SUB, st * WIN], dt.bfloat16, kind="ExternalInput"
    ).ap()
    t_W2p = nc.dram_tensor("W2p", [HP, 256], dt.bfloat16, kind="ExternalInput").ap()
    t_out = nc.dram_tensor(
        "OUT", [128, nt * F], dt.float32, kind="ExternalOutput"
    ).ap()

    const = ctx.enter_context(tc.tile_pool(name="const", bufs=1))
    hbuf = ctx.enter_context(tc.tile_pool(name="hbuf", bufs=2))
    sgb = ctx.enter_context(tc.tile_pool(name="sgb", bufs=2))
    xsb = ctx.enter_context(tc.tile_pool(name="xsb", bufs=2))
    wtsb = ctx.enter_context(tc.tile_pool(name="wtsb", bufs=3))
    prodb = ctx.enter_context(tc.tile_pool(name="prodb", bufs=4))
    sl2p = ctx.enter_context(tc.tile_pool(name="sl2p", bufs=2))

    ps_w = ctx.enter_context(tc.tile_pool(name="ps_w", bufs=3, space="PSUM"))
    ps_z = ctx.enter_context(tc.tile_pool(name="ps_z", bufs=2, space="PSUM"))

    nc.gpsimd.load_library(library_config.mlp)

    # constants
    w2p = const.tile([HP, 256], dt.bfloat16)
    nc.sync.dma_start(w2p[:], t_W2p)

    mult = mybir.AluOpType.mult
    t0 = 0  # first tile of the current group
    for g, GBg in enumerate(gsizes):
        SGC = GBg * 4  # subtiles in this group
        hp8 = hbuf.tile([HP, GBg, TILE], dt.bfloat16, tag="hp")
        nc.sync.dma_start(
            hp8[:],
            t_hp[:, t0 * TILE : (t0 + GBg) * TILE].rearrange(
                "p (a e) -> p a e", e=TILE
            ),
        )
        sgt = sgb.tile([128, SGC, WIN], dt.bfloat16, tag="sg")
        nc.sync.dma_start(
            sgt[:].rearrange("p s w -> p (s w)"),
            t_seg[:, t0 * 4 * WIN : (t0 * 4 + SGC) * WIN],
        )
        xst = xsb.tile([128, SGC, F], dt.bfloat16, tag="xs")
        nc.sync.dma_start(
            xst[:].rearrange("p s i -> p (s i)"),
            t_xs[:, t0 * 4 * F : (t0 * 4 + SGC) * F],
        )
        sl2 = sl2p.tile([128, GBg, F], dt.float32, tag="sl2")
        for tt in range(GBg):
            ti = t0 + tt  # 512-edge tile index
            wt4 = ps_w.tile([128, 4, 256], dt.float32, tag="wt")
            for q in range(4):
                nc.tensor.matmul(
                    wt4[:, q, :],
                    hp8[:, tt, q * 128 : (q + 1) * 128],
                    w2p[:],
                    start=True,
                    stop=True,
                )
            # drain W^T to SBUF bf16 (ACT main + DVE sliver) so the product
            # runs in DVE 2x mode and GPSIMD can take a share (GPSIMD
            # cannot touch PSUM)
            wts = wtsb.tile([128, 4, 256], dt.bfloat16, tag="wts")
            nc.scalar.activation(
                wts[:, :, 0:XSP], wt4[:, :, 0:XSP],
                mybir.ActivationFunctionType.Copy,
            )
            nc.vector.tensor_copy(wts[:, :, XSP:256], wt4[:, :, XSP:256])
            pr4 = prodb.tile([128, 4, F, F], dt.bfloat16, tag="pr")
            wtv = wts[:].rearrange("p a (o i) -> p a o i", i=F)
            xsv = xst[:, 4 * tt : 4 * tt + 4, :].rearrange(
                "p a (one i) -> p a one i", one=1
            )
            DQ = 4 - PQ
            OS = F - POSP
            if POSP:
                nc.vector.tensor_tensor(
                    pr4[:, 0 : DQ - 1],
                    wtv[:, 0 : DQ - 1],
                    xsv[:, 0 : DQ - 1].broadcast_to([128, DQ - 1, F, F]),
                    mult,
                )
                nc.vector.tensor_tensor(
                    pr4[:, DQ - 1 : DQ, 0:OS],
                    wtv[:, DQ - 1 : DQ, 0:OS],
                    xsv[:, DQ - 1 : DQ].broadcast_to([128, 1, OS, F]),
                    mult,
                )
                nc.gpsimd.tensor_tensor(
                    pr4[:, DQ - 1 : DQ, OS:F],
                    wtv[:, DQ - 1 : DQ, OS:F],
                    xsv[:, DQ - 1 : DQ].broadcast_to([128, 1, POSP, F]),
                    mult,
                )
            else:
                nc.vector.tensor_tensor(
                    pr4[:, 0:DQ],
                    wtv[:, 0:DQ],
                    xsv[:, 0:DQ].broadcast_to([128, DQ, F, F]),
                    mult,
                )
            if PQ:
                nc.gpsimd.tensor_tensor(
                    pr4[:, DQ:4],
                    wtv[:, DQ:4],
                    xsv[:, DQ:4].broadcast_to([128, PQ, F, F]),
                    mult,
                )
            if tt % 2 == 0:
                z8 = ps_z.tile([128, 2, 256], dt.float32, tag="z")
            for q in range(4):
                nc.tensor.matmul(
                    z8[32 * q : 32 * q + 32, tt % 2, :],
                    sgt[:, 4 * tt + q, :],
                    pr4[:, q, :, :].rearrange("p o i -> p (o i)"),
                    start=True,
                    stop=True,
                    tile_position=(0, 32 * q),
                )
            if tt % 2 == 1:
                nc.vector.tensor_reduce(
                    sl2[:, tt - 1 : tt + 1, :],
                    z8[:].rearrange("p a (o i) -> p a o i", i=F),
                    mybir.AxisListType.X,
                    mybir.AluOpType.add,
                )
        # stream this group's per-slot sums out; host does the node
        # permutation and the *A + B finalize
        nc.sync.dma_start(
            t_out[:, t0 * F : (t0 + GBg) * F],
            sl2[:].rearrange("p a c -> p (a c)"),
        )
        t0 += GBg


_CACHE = {}


PARTS = os.environ.get("BASS_GNN_PARTS", "v2")


def _get_program(ep: int):
    key = (ep, PARTS)
    if key not in _CACHE:
        nc = bacc.Bacc("TRN2", target_bir_lowering=False, debug=False)
        with tile.TileContext(nc) as tc:
            if "v2" in PARTS.split(","):
                _build_tile_kernel2(tc, ep, parts=PARTS)
            else:
                _build_tile_kernel(tc, ep, parts=PARTS)
        nc.compile()
        _CACHE[key] = nc
    return _CACHE[key]


LAST_RESULTS = None


def kernel(x, edge_feat, src, dst, W1, b1, W2, b2, bias):
    global LAST_RESULTS
    v2 = "v2" in PARTS.split(",")
    if v2:
        in_maps, ep, nbounds, finals = _host_prep2(
            x, edge_feat, src, dst, W1, b1, W2, b2, bias
        )
    else:
        in_maps, ep, _ = _host_prep(x, edge_feat, src, dst, W1, b1, W2, b2, bias)
    nc = _get_program(ep)
    trace = os.environ.get("BASS_GNN_TRACE", "") == "1"
    res = run_bass_kernel_spmd(nc, in_maps, list(range(NCORES)), trace=trace)
    LAST_RESULTS = res
    out = np.empty((N_NODES, F), np.float32)
    for c in range(NCORES):
        o = res.results[c]["OUT"]
        if v2:
            nt = ep // TILE
            slot_row, slot_col, A, B = finals[c]
            sl2 = o.reshape(128, nt, F)
            msum = sl2[slot_row, slot_col]
            out[nbounds[c] : nbounds[c + 1]] = msum * A + B
        else:
            out[c * NPC : (c + 1) * NPC] = o[:NPC]
    return out



# revision 66
# speedup vs baseline: 1.0400x; 1.0103x over previous
"""DynamicGraphConv Trainium2 kernel (8 NeuronCores, SPMD).

Edges are sorted by destination and sharded across cores at node-aligned
equal-edge boundaries, so each core owns a disjoint slice of the output
and no cross-core collective is needed.

Host prep (index bookkeeping + first MLP layer):
  - h = relu(edge_feat @ W1 + b1) computed host-side, shipped as
    hp^T [65, ep] bf16 (row 64 = ones, folding b2 via W2p's extra row).
  - x[src] gathered per-edge, shipped edge-major ([128, st, 16] bf16).
  - Edge runs (one per destination node) are bin-packed (FFD over degree
    buckets) into 128-edge subtiles, <=32 node-slots each; the one-hot
    seg matrices [128, st, 32] bf16 are built host-side.
  - W2 columns permuted to (o, i) order; b2 appended as row 64.

Per-core device pipeline (v2, edge-major, 512-edge tiles = 4 subtiles):
  mm2:   wt[128e, 256] = hp[:,q].T @ W2p          (PE x4, PSUM fp32)
  drain: wts = bf16(wt)                           (ACT cols<240, DVE rest)
  prod:  pr[e,(o,i)] = wts * xs[e,i]              (DVE 2x q0-2, GPSIMD q3;
         xs broadcast along o via stride-0 AP — no replication DMA)
  segmm: z[32q:32q+32, 256] = seg_q.T @ pr_q      (PE x4, one PSUM bank
         per 2 tiles, tile_position column offsets)
  red:   sl2[:, tile, :] = sum_i z                (DVE tensor_reduce X,
         node-level: 10x fewer rows than edges)
  out:   sl2 [128, 8, 16] streamed to DRAM per group.

Host finalize: out[n] = sl2[slot(n)] * A[n] + B[n] (pure permutation;
A = 1/max(cnt,1) masked, B = bias or x[n]+bias).
"""

import os
import sys

import numpy as np

for _p in ("/opt/trn_rl_repo",):
    if _p not in sys.path and os.path.isdir(_p):
        sys.path.insert(0, _p)

import ml_dtypes  # noqa: E402

import concourse.bass as bass  # noqa: E402
import concourse.tile as tile  # noqa: E402
from concourse import bacc  # noqa: E402
from concourse import library_config  # noqa: E402
from concourse import mybir  # noqa: E402
from concourse._compat import with_exitstack  # noqa: E402
from concourse.bass_utils import run_bass_kernel_spmd  # noqa: E402

dt = mybir.dt

N_NODES = 50000
N_EDGES = 500000
F = 16
H = 64
HP = H + 1  # hidden + b2 row
NCORES = 8
NPC = N_NODES // NCORES  # 6250 nodes per core
NPAD = 6272  # 49 * 128
SACR = NPAD - 1  # sacrificial node row
TILE = 512
SUB = 128  # edges per segment subtile
WIN = 32  # node window per subtile
ST_ALIGN = 64  # subtiles per PSUM-bank batch


def _pack_core(dloc):
    """Greedy-pack sorted local-dst edges into 128-edge subtiles such that
    no node's run crosses a subtile boundary and each subtile spans < WIN
    nodes.  Returns (pos[e] -> padded position, b[t] window bases,
    Tn{node->subtile})."""
    ne = len(dloc)
    pos = np.empty(ne, np.int64)
    b = []
    Tn = {}
    t = -1
    fill = SUB  # force new subtile at first node
    base = -1
    i = 0
    while i < ne:
        n = dloc[i]
        j = i
        while j < ne and dloc[j] == n:
            j += 1
        d = j - i
        assert d <= SUB, f"degree {d} exceeds subtile size"
        if (SUB - fill) < d or (n - base) >= WIN:
            t += 1
            fill = 0
            base = n
            b.append(n)
        Tn[n] = t
        pos[i:j] = t * SUB + fill + np.arange(d)
        fill += d
        i = j
    return pos, np.array(b, np.int64), Tn


def _host_prep(x, edge_feat, src, dst, W1, b1, W2, b2, bias):
    x = np.asarray(x, np.float32)
    edge_feat = np.asarray(edge_feat, np.float32)
    src = np.asarray(src, np.int64)
    dst = np.asarray(dst, np.int64)
    W1 = np.asarray(W1, np.float32)
    b1 = np.asarray(b1, np.float32)
    W2 = np.asarray(W2, np.float32)
    b2 = np.asarray(b2, np.float32)
    bias = np.asarray(bias, np.float32)

    order = np.argsort(dst, kind="stable")
    dst_s = dst[order]
    src_s = src[order]
    bounds = np.searchsorted(dst_s, np.arange(NCORES + 1) * NPC)

    packs = []
    st_max = 0
    for c in range(NCORES):
        lo, hi = bounds[c], bounds[c + 1]
        dloc = dst_s[lo:hi] - c * NPC
        pos, b, Tn = _pack_core(dloc)
        packs.append((pos, b, Tn))
        st_max = max(st_max, len(b))
    st = int(np.ceil(st_max / ST_ALIGN) * ST_ALIGN)
    ep = st * SUB

    # shared constants
    W1z = np.concatenate([W1, np.zeros((F, 1), np.float32)], axis=1)  # [16,65]
    biasv = np.concatenate([b1, np.ones(1, np.float32)])[:, None]  # [65,1]
    W2p = np.concatenate([W2, b2[None, :]], axis=0)  # [65,256]
    Ired = np.zeros((128, 2, F), np.float32)
    for c2 in range(2):
        for m in range(128):
            Ired[m, c2, (c2 * 128 + m) % F] = 1.0
    Ired = Ired.astype(ml_dtypes.bfloat16)
    iota64 = np.broadcast_to(
        np.arange(WIN, dtype=np.float32), (128, WIN)
    ).copy()

    cnt_all = np.bincount(dst_s, minlength=N_NODES).astype(np.float32)

    in_maps = []
    for c in range(NCORES):
        lo, hi = bounds[c], bounds[c + 1]
        ec = hi - lo
        pos, b, Tn = packs[c]
        nst = len(b)

        efdt_np = (
            ml_dtypes.bfloat16
            if "bf16mm1"
            in os.environ.get("BASS_GNN_PARTS", "all,gb8,bf16mm1").split(",")
            else np.float32
        )
        efT = np.zeros((F, ep), efdt_np)
        xsTb = np.zeros((F, ep), ml_dtypes.bfloat16)
        efT[:, pos] = edge_feat[order[lo:hi]].T.astype(efdt_np)
        xsTb[:, pos] = x[src_s[lo:hi]].T.astype(ml_dtypes.bfloat16)

        dloc = dst_s[lo:hi] - c * NPC
        dlr = np.full((ep,), -1.0, np.float32)
        dlr[pos] = (dloc - b[(pos // SUB)]).astype(np.float32)
        assert dlr.max() < WIN
        dlocrel = dlr.reshape(st, SUB).T.copy()  # [128, st] f32

        # scatter-entry position for (t, w): the PSUM bank/slab2 layout puts
        # subtile t at partition 64*(t%2)+w, col-group (t//2)%32, bank t//64
        idx2 = np.full((st * 64,), SACR, np.int64)
        for n, t in Tn.items():
            w = n - b[t]
            i = (t // 64) * 4096 + ((t // 2) % 32) * 128 + 64 * (t % 2) + w
            idx2[i] = n
        n2 = st * 64
        idxw = np.zeros((16, n2 // 16), np.int16)
        idxw[np.arange(n2) % 16, np.arange(n2) // 16] = idx2.astype(np.int16)
        idxw = np.tile(idxw, (8, 1))

        cnt = np.zeros((NPAD,), np.float32)
        cnt[:NPC] = cnt_all[c * NPC : (c + 1) * NPC]
        A = np.where(cnt > 0, 1.0 / np.maximum(cnt, 1.0), 0.0).astype(np.float32)
        B = np.broadcast_to(bias, (NPAD, F)).copy().astype(np.float32)
        nomsg = cnt[:NPC] == 0
        if nomsg.any():
            xn = x[c * NPC : (c + 1) * NPC]
            B[:NPC][nomsg] = xn[nomsg] + bias

        in_maps.append(
            {
                "efT": efT,
                "xsTb": xsTb,
                "dlocrel": dlocrel,
                "idx2": idxw,
                "A": A[:, None],
                "B": B,
                "W1z": W1z.astype(efdt_np),
                "biasv": biasv,
                "W2p": W2p,
                "Ired": Ired,
                "iota64": iota64,
            }
        )
    return in_maps, ep, bounds


@with_exitstack
def _build_tile_kernel(ctx, tc: tile.TileContext, ep: int, parts: str = "all"):
    has = lambda p: ("all" in parts.split(",") and p in ("main","msg","seg","scat","fin")) or p in parts.split(",")
    rep = 1
    for tok in parts.split(","):
        if tok.startswith("rep"):
            rep = int(tok[3:])
    nc = tc.nc
    st = ep // SUB
    mmdt = dt.float32r if "f32r" in parts.split(",") else dt.float32
    efdt = dt.bfloat16 if "bf16mm1" in parts.split(",") else mmdt
    t_efT = nc.dram_tensor("efT", [F, ep], efdt, kind="ExternalInput").ap()
    t_xsTb = nc.dram_tensor("xsTb", [F, ep], dt.bfloat16, kind="ExternalInput").ap()
    t_dlr = nc.dram_tensor(
        "dlocrel", [128, st], dt.float32, kind="ExternalInput"
    ).ap()
    t_idx2 = nc.dram_tensor(
        "idx2", [128, (st * 64) // 16], dt.int16, kind="ExternalInput"
    ).ap()
    t_A = nc.dram_tensor("A", [NPAD, 1], dt.float32, kind="ExternalInput").ap()
    t_B = nc.dram_tensor("B", [NPAD, F], dt.float32, kind="ExternalInput").ap()
    t_W1z = nc.dram_tensor("W1z", [F, HP], efdt, kind="ExternalInput").ap()
    t_biasv = nc.dram_tensor("biasv", [HP, 1], dt.float32, kind="ExternalInput").ap()
    t_W2p = nc.dram_tensor("W2p", [HP, 256], mmdt, kind="ExternalInput").ap()
    t_Ired = nc.dram_tensor(
        "Ired", [128, 2, F], dt.bfloat16, kind="ExternalInput"
    ).ap()
    t_iota = nc.dram_tensor(
        "iota64", [128, WIN], dt.float32, kind="ExternalInput"
    ).ap()
    t_out = nc.dram_tensor("OUT", [NPAD, F], dt.float32, kind="ExternalOutput").ap()
    nb = nc.dram_tensor("node_buf", [NPAD, 64], dt.float32).ap()

    const = ctx.enter_context(tc.tile_pool(name="const", bufs=1))
    deep = 1 if "deep" in parts.split(",") else 0
    gb8 = "gb8" in parts.split(",")
    ebuf = ctx.enter_context(tc.tile_pool(name="ebuf", bufs=2 if gb8 else 3 + deep))
    hbuf = ctx.enter_context(tc.tile_pool(name="hbuf", bufs=2 + deep))
    pbuf = ctx.enter_context(tc.tile_pool(name="pbuf", bufs=3 if gb8 else 4 + 2 * deep))
    mbuf = ctx.enter_context(tc.tile_pool(name="mbuf", bufs=3 + deep))
    segb = ctx.enter_context(tc.tile_pool(name="segb", bufs=6))
    slab2 = ctx.enter_context(tc.tile_pool(name="slab2", bufs=1))
    fin = ctx.enter_context(tc.tile_pool(name="fin", bufs=3))
    ps_h = ctx.enter_context(tc.tile_pool(name="ps_h", bufs=2, space="PSUM"))
    ps_w = ctx.enter_context(tc.tile_pool(name="ps_w", bufs=2, space="PSUM"))
    ps_m = ctx.enter_context(tc.tile_pool(name="ps_m", bufs=2, space="PSUM"))
    ps_s = ctx.enter_context(tc.tile_pool(name="ps_s", bufs=2, space="PSUM"))

    nc.gpsimd.load_library(library_config.mlp)

    # constants
    w1z = const.tile([F, HP], efdt)
    nc.sync.dma_start(w1z[:], t_W1z)
    bv = const.tile([HP, 1], dt.float32)
    nc.sync.dma_start(bv[:], t_biasv)
    w2p = const.tile([HP, 256], mmdt)
    nc.sync.dma_start(w2p[:], t_W2p)
    ired = const.tile([128, 2, F], dt.bfloat16)
    nc.sync.dma_start(ired[:], t_Ired)
    iota = const.tile([128, WIN], dt.float32)
    nc.sync.dma_start(iota[:], t_iota)
    dlrt = const.tile([128, st], dt.float32)
    nc.sync.dma_start(dlrt[:], t_dlr)
    idxt = const.tile([128, (st * 64) // 16], dt.int16)
    nc.sync.dma_start(idxt[:], t_idx2)
    sl2 = slab2.tile([128, st // 2, F], dt.float32)
    nc.vector.memset(sl2[:], 0.0)

    # zero the node accumulator (batched: 1024 rows per DMA)
    JB = max(j for j in range(1, 8) if NPAD % (128 * j) == 0)
    zt = const.tile([128, JB, 64], dt.float32)
    nc.vector.memset(zt[:], 0.0)

    rep_cm = tc.For_i(0, rep, 1, name="rep") if rep > 1 else None
    if rep_cm is not None:
        ctx.enter_context(rep_cm)

    nbz = nb.rearrange("(t j p) c -> t p j c", p=128, j=JB)
    for t in range(NPAD // (128 * JB)):
        nc.sync.dma_start(nbz[t], zt[:])

    GB = 8 if "gb8" in parts.split(",") else 4  # tiles per DMA group
    nt = ep // TILE if has("main") else 0
    assert nt % GB == 0 or nt == 0
    for g in range(nt // GB):
        gs = slice(g * GB * TILE, (g + 1) * GB * TILE)
        eft4 = ebuf.tile([F, GB, TILE], efdt, tag="eft")
        nc.sync.dma_start(eft4[:], t_efT[:, gs].rearrange("p (a e) -> p a e", e=TILE))
        xsb4 = ebuf.tile([F, GB * TILE], dt.bfloat16, tag="xsb")
        nc.sync.dma_start(xsb4[:], t_xsTb[:, gs])
        xr4 = [None, None]
        for c in range(2):
            xr4[c] = pbuf.tile([128, GB, TILE], dt.bfloat16, tag=f"xr{c}", name=f"xr{c}")
            nc.sync.dma_start(
                xr4[c][:].rearrange("p a e -> p (a e)"),
                xsb4[c * 8 : (c + 1) * 8, :]
                .rearrange("p (one e) -> p one e", one=1)
                .broadcast_to([8, 16, GB * TILE]),
            )
        mt4 = mbuf.tile([F, GB, TILE], dt.bfloat16, tag="msgT")

        for tt in range(GB):
            t = g * GB + tt
            hpre = ps_h.tile([HP, TILE], dt.float32)
            nc.tensor.matmul(
                hpre[:], w1z[:], eft4[:, tt, :], start=True, stop=True
            )
            hp = hbuf.tile([HP, TILE], mmdt, tag="hp")
            nc.scalar.activation(
                hp[:], hpre[:], mybir.ActivationFunctionType.Relu, bias=bv[:]
            )

            mg = ps_m.tile([F, TILE], dt.float32, tag="mg")
            for c in range(2):
                wt = ps_w.tile([128, TILE], dt.float32, tag="wt")
                nc.tensor.matmul(
                    wt[:], w2p[:, c * 128 : (c + 1) * 128], hp[:],
                    start=True, stop=True,
                )
                pr = pbuf.tile([128, TILE], dt.bfloat16, tag="prod")
                nc.vector.tensor_tensor(
                    pr[:], wt[:], xr4[c][:, tt, :], mybir.AluOpType.mult
                )
                nc.tensor.matmul(
                    mg[:], ired[:, c, :], pr[:], start=(c == 0), stop=(c == 1)
                )

            if has("msg"):
                nc.scalar.activation(
                    mt4[:, tt, :], mg[:], mybir.ActivationFunctionType.Copy
                )

        if not has("msg"):
            continue
        tr4 = mbuf.tile([128, GB * (TILE // SUB), F], dt.bfloat16, tag="tr")
        nc.sync.dma_start(
            tr4[:], mt4[:].rearrange("p a e -> p (a e)"), transpose=True
        )

        if not has("seg"):
            continue
        for q in range(GB * (TILE // SUB)):
            stile = g * GB * (TILE // SUB) + q
            seg = segb.tile([128, WIN], dt.bfloat16, tag="seg")
            nc.gpsimd.tensor_scalar(
                seg[:],
                iota[:],
                dlrt[:, stile : stile + 1],
                None,
                mybir.AluOpType.is_equal,
            )
            if stile % ST_ALIGN == 0:
                bank = ps_s.tile([128, 512], dt.float32, tag="bank")
            prow = 64 * (stile % 2)
            pcol = ((stile // 2) % (ST_ALIGN // 2)) * F
            nc.tensor.matmul(
                bank[prow : prow + WIN, pcol : pcol + F],
                seg[:],
                tr4[:, q, :],
                start=True,
                stop=True,
            )
            if stile % ST_ALIGN == ST_ALIGN - 1:
                gg = stile // ST_ALIGN
                gsl = slice(gg * (ST_ALIGN // 2), (gg + 1) * (ST_ALIGN // 2))
                for half in range(2):
                    nc.vector.tensor_copy(
                        sl2[64 * half : 64 * half + WIN, gsl, :],
                        bank[64 * half : 64 * half + WIN, :].rearrange(
                            "p (a b) -> p a b", b=F
                        ),
                    )

    # scatter in ring-sized chunks (disjoint real rows -> safe unordered)
    csz = 4096
    for o in range(0, st * 64 if has("scat") else 0, csz):
        n_i = min(csz, st * 64 - o)
        nc.gpsimd.dma_scatter_add(
            nb[:, 0:F],
            sl2[:, o // 128 : (o + n_i) // 128, :],
            idxt[:, o // 16 : (o + n_i) // 16],
            n_i,
            n_i,
            F,
            elem_step=64,
        )

    # finalize: out = msum * A + B  (batched 512 rows per step)
    FB = 4
    nfin = NPAD // 128 if has("fin") else 0
    n = 0
    while n < nfin:
        k = min(FB, nfin - n)
        r = slice(n * 128, (n + k) * 128)
        nbt = fin.tile([128, FB, F], dt.float32, tag="nbt")
        nc.sync.dma_start(
            nbt[:, :k, :], nb[r, 0:F].rearrange("(j p) c -> p j c", p=128)
        )
        at = fin.tile([128, FB, 1], dt.float32, tag="at")
        nc.sync.dma_start(at[:, :k, :], t_A[r].rearrange("(j p) c -> p j c", p=128))
        bt = fin.tile([128, FB, F], dt.float32, tag="bt")
        nc.sync.dma_start(bt[:, :k, :], t_B[r].rearrange("(j p) c -> p j c", p=128))
        ot = fin.tile([128, FB, F], dt.float32, tag="ot")
        abc = at[:, :k, :].broadcast_to([128, k, F])
        nc.vector.tensor_tensor(ot[:, :k, :], nbt[:, :k, :], abc,
                                mybir.AluOpType.mult)
        nc.vector.tensor_tensor(ot[:, :k, :], ot[:, :k, :], bt[:, :k, :],
                                mybir.AluOpType.add)
        nc.sync.dma_start(t_out[r].rearrange("(j p) c -> p j c", p=128), ot[:, :k, :])
        n += k


def _pack_core2(dloc):
    """Bin-pack node edge-runs into 128-edge subtiles (<=WIN nodes each)
    with first-fit-decreasing over integer degree buckets. Node->slot
    assignment is free-form (the one-hot seg matrices and scatter indices
    are host-built), so no window-locality constraint applies.

    Returns (pos[e] -> packed position, slot_of_node {n: (t, w)})."""
    ne = len(dloc)
    nodes, starts, degs = np.unique(dloc, return_index=True, return_counts=True)
    assert degs.max() <= SUB
    maxd = int(degs.max())
    # degree buckets: list of node indices per degree
    buckets = [[] for _ in range(maxd + 1)]
    for ni, d in enumerate(degs):
        buckets[d].append(ni)
    remaining = len(nodes)
    bins = []
    while remaining:
        # new bin: repeatedly take the largest-degree node that still fits
        rem, slots, members = SUB, WIN, []
        while slots:
            d = min(rem, maxd)
            while d > 0 and not buckets[d]:
                d -= 1
            if d == 0:
                break
            members.append(buckets[d].pop())
            rem -= d
            slots -= 1
            remaining -= 1
        bins.append(members)
    pos = np.empty(ne, np.int64)
    slot_of_node = {}
    for t, members in enumerate(bins):
        fill = 0
        for w, ni in enumerate(members):
            d = degs[ni]
            s0 = starts[ni]
            pos[s0 : s0 + d] = t * SUB + fill
            pos[s0 : s0 + d] += np.arange(d)
            fill += d
            slot_of_node[nodes[ni]] = (t, w)
    return pos, slot_of_node, len(bins)


def _host_prep2(x, edge_feat, src, dst, W1, b1, W2, b2, bias):
    """Host prep for the edge-major v2 pipeline.

    v2 layout notes:
      - W2 columns are permuted to (o, i) order (c = o*16 + i) so the
        per-edge i-reduction is over the innermost axis.
      - xs is shipped edge-major: partition p of subtile s holds
        x[src[edge s*128+p]][:] as 16 contiguous bf16 values.
      - Z/scatter slot mapping: subtile t window-slot w sits at scatter row
        j = (t//128)*4096 + ((t//4)%32)*128 + 32*(t%4) + w, matching a
        [128, st//4, 16] SBUF accumulator with 4 subtiles per 128
        partitions and one 16-wide column group per 512-edge tile.
    """
    x = np.asarray(x, np.float32)
    edge_feat = np.asarray(edge_feat, np.float32)
    src = np.asarray(src, np.int64)
    dst = np.asarray(dst, np.int64)
    W1 = np.asarray(W1, np.float32)
    b1 = np.asarray(b1, np.float32)
    W2 = np.asarray(W2, np.float32)
    b2 = np.asarray(b2, np.float32)
    bias = np.asarray(bias, np.float32)

    order = np.argsort(dst, kind="stable")
    dst_s = dst[order]
    src_s = src[order]
    # equal-edge sharding with node-aligned boundaries
    cum = np.cumsum(np.bincount(dst_s, minlength=N_NODES))
    nbounds = np.concatenate(
        [
            [0],
            np.searchsorted(cum, (np.arange(1, NCORES) * len(dst_s)) // NCORES),
            [N_NODES],
        ]
    ).astype(np.int64)
    bounds = np.searchsorted(dst_s, nbounds)

    packs = []
    st_max = 0
    for c in range(NCORES):
        lo, hi = bounds[c], bounds[c + 1]
        dloc = dst_s[lo:hi] - nbounds[c]
        pos, slot_of_node, nst = _pack_core2(dloc)
        packs.append((pos, slot_of_node))
        st_max = max(st_max, nst)
    st = int(np.ceil(st_max / 4) * 4)
    ep = st * SUB

    # shared constants
    W2p = np.concatenate([W2, b2[None, :]], axis=0)  # [65,256] cols (i,o)
    # permute cols to (o, i): new col o*16+i <- old col i*16+o
    oc = np.arange(256)
    W2perm = W2p[:, (oc % 16) * 16 + (oc // 16)].astype(ml_dtypes.bfloat16)

    # per-edge hidden activations computed host-side (first MLP layer)
    h_all = np.maximum(edge_feat @ W1 + b1, 0.0).astype(ml_dtypes.bfloat16)

    cnt_all = np.bincount(dst_s, minlength=N_NODES).astype(np.float32)

    in_maps = []
    finals = []
    for c in range(NCORES):
        lo, hi = bounds[c], bounds[c + 1]
        pos, slot_of_node = packs[c]

        # hp feature-major [65, ep]: rows 0..63 = h.T, row 64 = ones
        hpT = np.zeros((HP, ep), ml_dtypes.bfloat16)
        hpT[:H, pos] = h_all[order[lo:hi]].T
        hpT[H, :] = 1.0

        xs = np.zeros((ep, F), ml_dtypes.bfloat16)
        xs[pos] = x[src_s[lo:hi]].astype(ml_dtypes.bfloat16)
        # [128, st*16]: partition p, free s*16+i  <-  edge s*128+p feature i
        xs_em = np.ascontiguousarray(
            xs.reshape(st, SUB, F).transpose(1, 0, 2).reshape(SUB, st * F)
        )

        # one-hot seg matrices from the free-form slot assignment
        dloc = dst_s[lo:hi] - nbounds[c]
        wof = np.array(
            [slot_of_node[n][1] for n in dloc], dtype=np.int64
        )
        segf = np.zeros((ep, WIN), ml_dtypes.bfloat16)
        segf[pos, wof] = 1.0
        seg_all = np.ascontiguousarray(
            segf.reshape(st, SUB, WIN).transpose(1, 0, 2).reshape(SUB, st * WIN)
        )

        # host-side finalize data: sl2 slot of each local node + A, B
        # sl2 layout: partition 32*(t%4)+w, column t//4
        nlo, nhi = nbounds[c], nbounds[c + 1]
        nloc = nhi - nlo
        slot_row = np.zeros((nloc,), np.int64)
        slot_col = np.zeros((nloc,), np.int64)
        for n, (t, w) in slot_of_node.items():
            slot_row[n] = 32 * (t % 4) + w
            slot_col[n] = t // 4
        cnt = cnt_all[nlo:nhi]
        A = np.where(cnt > 0, 1.0 / np.maximum(cnt, 1.0), 0.0).astype(np.float32)
        B = np.broadcast_to(bias, (nloc, F)).copy().astype(np.float32)
        nomsg = cnt == 0
        if nomsg.any():
            xn = x[nlo:nhi]
            B[nomsg] = xn[nomsg] + bias

        in_maps.append(
            {
                "hpT": hpT,
                "xs_em": xs_em,
                "seg_all": seg_all,
                "W2p": W2perm,
            }
        )
        finals.append((slot_row, slot_col, A[:, None], B))
    return in_maps, ep, nbounds, finals


@with_exitstack
def _build_tile_kernel2(ctx, tc: tile.TileContext, ep: int, parts: str = "v2"):
    """Edge-major pipeline: per 512-edge tile (4 x 128-edge subtiles)
      mm1:  hpre[65,512] = W1z.T @ efT            (PE, bf16, PSUM)
      relu: hp = relu(hpre + [b1;1]) -> bf16 SBUF (ACT)
      mm2:  wt[128e, 256] = hp[:,q].T @ W2perm    (PE x4, PSUM, cols (o,i))
      prod: pr = wt * xs_bcast  -> bf16 SBUF      (DVE o<OD, GPSIMD o>=OD)
      seg:  one-hot seg[128,32] (GPSIMD is_equal)
      segmm: z[32q:32q+32, 256] = seg.T @ pr      (PE x4, PSUM)
      red:  sl2[:, tile, :] = sum_i z             (DVE tensor_reduce X)
      scatter + finalize as v1.
    """
    toks = parts.split(",")
    PQ = 1  # product quarters on GPSIMD (reads SBUF wts), rest on DVE
    XSP = 236  # wt-drain split point: ACT copies cols [0,XSP), DVE the rest
    POSP = 0  # extra o-slices of the last DVE quarter shifted to GPSIMD
    for tk in toks:
        if tk.startswith("pq"):
            PQ = int(tk[2:])
        if tk.startswith("xsp"):
            XSP = int(tk[3:])
        if tk.startswith("po"):
            POSP = int(tk[2:])
    nc = tc.nc
    st = ep // SUB
    nt = ep // TILE  # 512-edge tiles
    GB = 8
    for tk in toks:
        if tk.startswith("gb"):
            GB = int(tk[2:])
    # group sizes: small first group shortens the pipeline head
    gsizes = []
    rem = nt
    if nt > GB and "nog02" not in toks:
        gsizes.append(2)
        rem -= 2
    while rem > 0:
        gsizes.append(min(GB, rem))
        rem -= gsizes[-1]

    t_hp = nc.dram_tensor("hpT", [HP, ep], dt.bfloat16, kind="ExternalInput").ap()
    t_xs = nc.dram_tensor(
        "xs_em", [SUB, st * F], dt.bfloat16, kind="ExternalInput"
    ).ap()
    t_seg = nc.dram_tensor(
        "seg_all", [SUB, st * WIN], dt.bfloat16, kind="ExternalInput"
    ).ap()
    t_W2p = nc.dram_tensor("W2p", [HP, 256], dt.bfloat16, kind="ExternalInput").ap()
    t_out = nc.dram_tensor(
        "OUT", [128, nt * F], dt.float32, kind="ExternalOutput"
    ).ap()

    const = ctx.enter_context(tc.tile_pool(name="const", bufs=1))
    hbuf = ctx.enter_context(tc.tile_pool(name="hbuf", bufs=2))
    sgb = ctx.enter_context(tc.tile_pool(name="sgb", bufs=2))
    xsb = ctx.enter_context(tc.tile_pool(name="xsb", bufs=2))
    wtsb = ctx.enter_context(tc.tile_pool(name="wtsb", bufs=3))
    prodb = ctx.enter_context(tc.tile_pool(name="prodb", bufs=4))
    sl2p = ctx.enter_context(tc.tile_pool(name="sl2p", bufs=2))

    ps_w = ctx.enter_context(tc.tile_pool(name="ps_w", bufs=3, space="PSUM"))
    ps_z = ctx.enter_context(tc.tile_pool(name="ps_z", bufs=2, space="PSUM"))

    nc.gpsimd.load_library(library_config.mlp)

    # constants
    w2p = const.tile([HP, 256], dt.bfloat16)
    nc.sync.dma_start(w2p[:], t_W2p)

    mult = mybir.AluOpType.mult
    t0 = 0  # first tile of the current group
    for g, GBg in enumerate(gsizes):
        SGC = GBg * 4  # subtiles in this group
        hp8 = hbuf.tile([HP, GBg, TILE], dt.bfloat16, tag="hp")
        nc.sync.dma_start(
            hp8[:],
            t_hp[:, t0 * TILE : (t0 + GBg) * TILE].rearrange(
                "p (a e) -> p a e", e=TILE
            ),
        )
        sgt = sgb.tile([128, SGC, WIN], dt.bfloat16, tag="sg")
        nc.sync.dma_start(
            sgt[:].rearrange("p s w -> p (s w)"),
            t_seg[:, t0 * 4 * WIN : (t0 * 4 + SGC) * WIN],
        )
        xst = xsb.tile([128, SGC, F], dt.bfloat16, tag="xs")
        nc.sync.dma_start(
            xst[:].rearrange("p s i -> p (s i)"),
            t_xs[:, t0 * 4 * F : (t0 * 4 + SGC) * F],
        )
        sl2 = sl2p.tile([128, GBg, F], dt.float32, tag="sl2")
        for tt in range(GBg):
            ti = t0 + tt  # 512-edge tile index
            wt4 = ps_w.tile([128, 4, 256], dt.float32, tag="wt")
            for q in range(4):
                nc.tensor.matmul(
                    wt4[:, q, :],
                    hp8[:, tt, q * 128 : (q + 1) * 128],
                    w2p[:],
                    start=True,
                    stop=True,
                )
            # drain W^T to SBUF bf16 (ACT main + DVE sliver) so the product
            # runs in DVE 2x mode and GPSIMD can take a share (GPSIMD
            # cannot touch PSUM)
            wts = wtsb.tile([128, 4, 256], dt.bfloat16, tag="wts")
            nc.scalar.activation(
                wts[:, :, 0:XSP], wt4[:, :, 0:XSP],
                mybir.ActivationFunctionType.Copy,
            )
            nc.vector.tensor_copy(wts[:, :, XSP:256], wt4[:, :, XSP:256])
            pr4 = prodb.tile([128, 4, F, F], dt.bfloat16, tag="pr")
            wtv = wts[:].rearrange("p a (o i) -> p a o i", i=F)
            xsv = xst[:, 4 * tt : 4 * tt + 4, :].rearrange(
                "p a (one i) -> p a one i", one=1
            )
            DQ = 4 - PQ
            OS = F - POSP
            if POSP:
                nc.vector.tensor_tensor(
                    pr4[:, 0 : DQ - 1],
                    wtv[:, 0 : DQ - 1],
                    xsv[:, 0 : DQ - 1].broadcast_to([128, DQ - 1, F, F]),
                    mult,
                )
                nc.vector.tensor_tensor(
                    pr4[:, DQ - 1 : DQ, 0:OS],
                    wtv[:, DQ - 1 : DQ, 0:OS],
                    xsv[:, DQ - 1 : DQ].broadcast_to([128, 1, OS, F]),
                    mult,
                )
                nc.gpsimd.tensor_tensor(
                    pr4[:, DQ - 1 : DQ, OS:F],
                    wtv[:, DQ - 1 : DQ, OS:F],
                    xsv[:, DQ - 1 : DQ].broadcast_to([128, 1, POSP, F]),
                    mult,
                )
            else:
                nc.vector.tensor_tensor(
                    pr4[:, 0:DQ],
                    wtv[:, 0:DQ],
                    xsv[:, 0:DQ].broadcast_to([128, DQ, F, F]),
                    mult,
                )
            if PQ:
                nc.gpsimd.tensor_tensor(
                    pr4[:, DQ:4],
                    wtv[:, DQ:4],
                    xsv[:, DQ:4].broadcast_to([128, PQ, F, F]),
                    mult,
                )
            zk = min(2, GBg - (tt - tt % 2))  # z-pair width (1 on odd tail)
            if tt % 2 == 0:
                z8 = ps_z.tile([128, zk, 256], dt.float32, tag="z")
            for q in range(4):
                nc.tensor.matmul(
                    z8[32 * q : 32 * q + 32, tt % 2, :],
                    sgt[:, 4 * tt + q, :],
                    pr4[:, q, :, :].rearrange("p o i -> p (o i)"),
                    start=True,
                    stop=True,
                    tile_position=(0, 32 * q),
                )
            if tt % 2 == zk - 1:
                nc.vector.tensor_reduce(
                    sl2[:, tt - (zk - 1) : tt + 1, :],
                    z8[:].rearrange("p a (o i) -> p a o i", i=F),
                    mybir.AxisListType.X,
                    mybir.AluOpType.add,
                )
        # stream this group's per-slot sums out; host does the node
        # permutation and the *A + B finalize
        nc.sync.dma_start(
            t_out[:, t0 * F : (t0 + GBg) * F],
            sl2[:].rearrange("p a c -> p (a c)"),
        )
        t0 += GBg


_CACHE = {}


PARTS = os.environ.get("BASS_GNN_PARTS", "v2")


def _get_program(ep: int):
    key = (ep, PARTS)
    if key not in _CACHE:
        nc = bacc.Bacc("TRN2", target_bir_lowering=False, debug=False)
        with tile.TileContext(nc) as tc:
            if "v2" in PARTS.split(","):
                _build_tile_kernel2(tc, ep, parts=PARTS)
            else:
                _build_tile_kernel(tc, ep, parts=PARTS)
        nc.compile()
        _CACHE[key] = nc
    return _CACHE[key]


LAST_RESULTS = None


def kernel(x, edge_feat, src, dst, W1, b1, W2, b2, bias):
    global LAST_RESULTS
    v2 = "v2" in PARTS.split(",")
    if v2:
        in_maps, ep, nbounds, finals = _host_prep2(
            x, edge_feat, src, dst, W1, b1, W2, b2, bias
        )
    else:
        in_maps, ep, _ = _host_prep(x, edge_feat, src, dst, W1, b1, W2, b2, bias)
    nc = _get_program(ep)
    trace = os.environ.get("BASS_GNN_TRACE", "") == "1"
    res = run_bass_kernel_spmd(nc, in_maps, list(range(NCORES)), trace=trace)
    LAST_RESULTS = res
    out = np.empty((N_NODES, F), np.float32)
    for c in range(NCORES):
        o = res.results[c]["OUT"]
        if v2:
            nt = ep // TILE
            slot_row, slot_col, A, B = finals[c]
            sl2 = o.reshape(128, nt, F)
            msum = sl2[slot_row, slot_col]
            out[nbounds[c] : nbounds[c + 1]] = msum * A + B
        else:
            out[c * NPC : (c + 1) * NPC] = o[:NPC]
    return out

